# revision 23
# baseline (speedup 1.0000x reference)
"""GroupedQueryAttention forward on 8 Trainium2 NeuronCores (Bass/Tile), v2.

Sharding (per spec hint): data-parallel over batch (B=2) x tensor-parallel
over KV-head groups (4 groups of 2 KV heads + their 8 query heads each).
Core c -> (batch b = c // 4, group g = c % 4).

v2 redesign vs baseline (573us):
  - scores row-tiled 2x: the two K=64 (per-kv-head) score matmuls run
    concurrently in disjoint PE row groups (tile_position (0,0)/(64,0)).
  - attn@V col-tiled 2x: the two M=64 AV matmuls run concurrently in
    disjoint PE col groups ((0,0)/(0,64)), both kv heads in one PSUM bank.
  - softmax denominators via 4 concurrent M=1 matmuls (ones stationary)
    col-tiled into one PSUM bank; reciprocal via DVE approx; broadcast to
    128 partitions via stride-0-source DMA; normalize fused into the PSUM
    evacuation tensor_tensor.
  - causal triangle trimming: diagonal-block matmuls/exps only cover the
    columns right of the diagonal (query >= key block start).
  - exp batched to FD=2*F per ACTIVATE from bf16 score PSUM banks.
  - mt-outer loop reuses kT/V stationaries across the 2 head pairs of a
    group; V transposed via DMA xbar instead of the PE.
  - software-pipelined emission: projection and o_proj matmul groups are
    interleaved into the attention loop as PE fillers so the tensor engine
    never idles while the scalar engine chews exps (keeps HAM at K=8/8).
  - o_proj partials written as bf16 (halves output DMA); host sums fp32.

All device compute bf16 with fp32 PSUM accumulation (bf16 PSUM for raw
scores only, pre-exp). Host pre-casts/pre-transposes x and pre-slices and
pair-reorders the weight shards.
"""

from collections import deque

import numpy as np

import concourse.bass as bass  # noqa: F401  (import keeps engine registry warm)
import concourse.mybir as mybir
import concourse.tile as tile
from concourse import bacc, bass_utils

# Problem shape (hardcoded per contract).
B, N, D = 2, 2048, 2048
NUM_HEADS = 32
NUM_KV_HEADS = 8
HD = 64
G = NUM_HEADS // NUM_KV_HEADS  # 4
N_CORES = 8
NT = D // 128                  # 16 contraction tiles
NCHUNK = 4                     # token chunks of 512
CH = 512

_CACHE = {}


def _build():
    nc = bacc.Bacc("TRN2", target_bir_lowering=False, debug=False,
                   num_devices=N_CORES)
    f32, bf16 = mybir.dt.float32, mybir.dt.bfloat16
    Exp = mybir.ActivationFunctionType.Exp

    xT = nc.dram_tensor("xT", [D, N], bf16, kind="ExternalInput")
    wq = nc.dram_tensor("wq", [D, 512], bf16, kind="ExternalInput")
    wk = nc.dram_tensor("wk", [D, 128], bf16, kind="ExternalInput")
    wv = nc.dram_tensor("wv", [D, 128], bf16, kind="ExternalInput")
    wo = nc.dram_tensor("wo", [512, D], bf16, kind="ExternalInput")
    msk = nc.dram_tensor("msk", [128, 128], bf16, kind="ExternalInput")
    part = nc.dram_tensor("part", [N, D], bf16, kind="ExternalOutput")

    with tile.TileContext(nc) as tc:
        with (
            tc.tile_pool(name="const", bufs=1) as cpool,
            tc.tile_pool(name="proj", bufs=1) as kpool,
            tc.tile_pool(name="work", bufs=2) as wpool,
            tc.tile_pool(name="ps_s", bufs=1, space="PSUM") as ps_s,
            tc.tile_pool(name="ps_pa", bufs=1, space="PSUM") as ps_pa,
            tc.tile_pool(name="ps_d", bufs=1, space="PSUM") as ps_d,
            tc.tile_pool(name="ps_proj", bufs=1, space="PSUM") as ps_proj,
        ):
            # ---- inputs / constants -------------------------------------
            wk_t = cpool.tile([128, NT * 128], bf16, tag="wk")
            nc.sync.dma_start(
                wk_t[:].rearrange("p (t o) -> p t o", t=NT),
                wk.ap().rearrange("(t p) o -> p t o", p=128))
            wv_t = cpool.tile([128, NT * 128], bf16, tag="wv")
            nc.sync.dma_start(
                wv_t[:].rearrange("p (t o) -> p t o", t=NT),
                wv.ap().rearrange("(t p) o -> p t o", p=128))
            # wq/wo/msk ride the scalar-issued DMA queue so they load in
            # parallel with the sync-queue xt chunks.
            wq_t = cpool.tile([128, NT * 512], bf16, tag="wq")
            nc.scalar.dma_start(
                wq_t[:].rearrange("p (t o) -> p t o", t=NT),
                wq.ap().rearrange("(t p) o -> p t o", p=128))
            wo_t = cpool.tile([128, 4 * D], bf16, tag="wo")
            nc.scalar.dma_start(
                wo_t[:].rearrange("p (t o) -> p t o", t=4),
                wo.ap().rearrange("(t p) o -> p t o", p=128))
            msk_t = cpool.tile([128, 128], bf16, tag="msk")
            nc.scalar.dma_start(msk_t[:], msk.ap()[:])
            # x^T loaded in 4 t-range chunks with full 4KB row descriptors;
            # chunk 0 rides the sync queue alone so the first projection
            # chain can start ~7us in, later chunks prefetch via swdge.
            xtc = []
            for q in range(4):
                xq = cpool.tile([128, 4 * N], bf16, tag=f"xtc{q}")
                eng = nc.sync if q == 0 else nc.gpsimd
                eng.dma_start(
                    xq[:].rearrange("p (t n) -> p t n", t=4),
                    xT.ap().rearrange("(t p) n -> p t n", p=128)
                    [:, 4 * q:4 * q + 4, :])
                xtc.append(xq)

            def xt_rhs(t, j):
                return xtc[t // 4][:, (t % 4) * N + j * CH:
                                   (t % 4) * N + (j + 1) * CH]
            ones1 = cpool.tile([128, 1], bf16, tag="ones1")
            nc.vector.memset(ones1[:], 1.0)
            onesb = cpool.tile([128, 64], f32, tag="onesb")
            nc.vector.memset(onesb[:], 1.0)
            # pre-warm the exp activation table while the DMAs run
            scr = cpool.tile([1, 8], f32, tag="scr")
            nc.vector.memset(scr[:], 0.0)
            nc.scalar.activation(scr[0:1, :], scr[0:1, :], Exp)

            # persistent projection outputs
            kt = [kpool.tile([128, CH], bf16, tag=f"kt{j}", name=f"kt{j}")
                  for j in range(NCHUNK)]
            v3 = [kpool.tile([128, 128], bf16, tag=f"v3_{m}", name=f"v3_{m}")
                  for m in range(N // 128)]
            qt = [[kpool.tile([128, CH], bf16, tag=f"qt{a}_{j}",
                              name=f"qt{a}_{j}")
                   for j in range(NCHUNK)] for a in range(4)]

            # ---- filler machinery ---------------------------------------
            fillers = deque()  # (emit_fn, est_pe_ns)

            def drain(budget_ns):
                spent = 0
                while fillers and spent < budget_ns:
                    fn, est = fillers.popleft()
                    fn()
                    spent += est

            def drain_all():
                while fillers:
                    fn, _ = fillers.popleft()
                    fn()

            # ---- projection thunks --------------------------------------
            def proj_chain(dst_evac, lhsT_col, j, n_chunks=4):
                """Returns thunk fns for one 16-deep contraction chain.

                lhsT_col(t) -> AP for the stationary tile;
                dst_evac(ps) emits the evacuation."""
                ps_box = {}

                def quarter(q):
                    def emit():
                        if q == 0:
                            ps_box["ps"] = ps_proj.tile(
                                [128, CH], f32, tag="proj", name="pps")
                        ps = ps_box["ps"]
                        for t in range(4 * q, 4 * q + 4):
                            nc.tensor.matmul(
                                ps[:], lhsT_col(t), xt_rhs(t, j),
                                start=(t == 0), stop=(t == NT - 1))
                        if q == 3:
                            dst_evac(ps)
                    return emit
                return [(quarter(q), 1400) for q in range(4)]

            def proj_thunks(j):
                th = []

                def kev(ps):
                    nc.vector.tensor_copy(kt[j][:], ps[:])
                th += proj_chain(kev, lambda t: wk_t[:, t * 128:(t + 1) * 128], j)

                def vev(ps):
                    vt_s = wpool.tile([128, CH], bf16, tag="vt", name="vt_s")
                    nc.vector.tensor_copy(vt_s[:], ps[:])
                    for s in range(4):
                        nc.sync.dma_start_transpose(
                            v3[4 * j + s][:], vt_s[:, s * 128:(s + 1) * 128])
                th += proj_chain(vev, lambda t: wv_t[:, t * 128:(t + 1) * 128], j)
                for a in range(4):
                    def qev(ps, a=a):
                        nc.vector.tensor_copy(qt[a][j][:], ps[:])
                    th += proj_chain(
                        qev,
                        lambda t, a=a: wq_t[:, t * 512 + a * 128:
                                            t * 512 + (a + 1) * 128], j)
                return th

            # ---- o_proj thunks ------------------------------------------
            def oproj_thunks(ci, an_tiles, tail=False):
                n0 = ci * CH
                th = []
                for nt_ in range(4):
                    for dc in range(4):
                        def emit(nt_=nt_, dc=dc):
                            # in the tail (post-attention) phase the score
                            # banks are free: rotate over them too so the
                            # PE never waits on a single bank's evacuation
                            if tail and (nt_ * 4 + dc) % 2:
                                po = ps_s.tile([128, 1024], f32,
                                               tag=f"s{(nt_ * 4 + dc) // 2 % 2}",
                                               name="po2")[:, 0:CH]
                            else:
                                po = ps_proj.tile([128, CH], f32, tag="proj",
                                                  name="po")
                            for a in range(4):
                                nc.tensor.matmul(
                                    po[:],
                                    an_tiles[a][:, nt_ * 128:(nt_ + 1) * 128],
                                    wo_t[:, a * D + dc * CH:
                                         a * D + (dc + 1) * CH],
                                    start=(a == 0), stop=(a == 3))
                            st = wpool.tile([128, CH], bf16, tag="st",
                                            bufs=3, name="st")
                            if (nt_ + dc) % 2 == 0:
                                nc.scalar.copy(st[:], po[:])
                            else:
                                nc.vector.tensor_copy(st[:], po[:])
                            nc.gpsimd.dma_start(
                                part.ap()[n0 + nt_ * 128:n0 + (nt_ + 1) * 128,
                                          dc * CH:(dc + 1) * CH],
                                st[:])
                        th.append((emit, 1400))
                return th

            # ---- attention ----------------------------------------------
            def attn_chunk(ci):
                n0 = ci * CH
                M = 4 * ci + 4
                an_tiles = []
                for g in range(2):
                    a0, a1 = 2 * g, 2 * g + 1
                    paA = ps_pa.tile([128, CH], f32, tag="paA", name="paA")
                    paB = ps_pa.tile([128, CH], f32, tag="paB", name="paB")
                    dn = ps_d.tile([128, CH], f32, tag="dn", name="dn")
                    pts = {}

                    def scores_exp(mt):
                        # score layout per head pair a: kv0 block at cols
                        # [0:F], kv1 block at cols [512:512+F] (bank 2);
                        # one FD=512+F exp covers both (cols [F:512] are
                        # junk for diagonal tiles and never read).
                        jmt, cmt = mt // 4, mt % 4
                        flo = max(0, (mt - 4 * ci) * 128)
                        F = CH - flo
                        # kv1 block always in bank 1: the two row-tiled
                        # score matmuls run concurrently and must not
                        # target the same PSUM bank (cols [F:CH] junk)
                        off = CH
                        pt_pair = []
                        for i, a in enumerate((a0, a1)):
                            ss = ps_s.tile([128, 1024], f32, tag=f"s{i}",
                                           name="ss")
                            nc.tensor.matmul(
                                ss[:, 0:F],
                                kt[jmt][0:64, cmt * 128:(cmt + 1) * 128],
                                qt[a][ci][0:64, flo:CH],
                                start=True, stop=True, tile_position=(0, 0))
                            nc.tensor.matmul(
                                ss[:, off:off + F],
                                kt[jmt][64:128, cmt * 128:(cmt + 1) * 128],
                                qt[a][ci][64:128, flo:CH],
                                start=True, stop=True, tile_position=(64, 0))
                            pt_ = wpool.tile([128, 1024], bf16, tag=f"pt{i}",
                                             name="pt")
                            nc.scalar.activation(pt_[:, 0:off + F],
                                                 ss[:, 0:off + F],
                                                 Exp, scale=0.125)
                            if mt >= 4 * ci:  # diagonal block: causal mask
                                nc.vector.tensor_mul(
                                    pt_[:, 0:128], pt_[:, 0:128], msk_t[:])
                                nc.vector.tensor_mul(
                                    pt_[:, off:off + 128],
                                    pt_[:, off:off + 128], msk_t[:])
                            pt_pair.append(pt_)
                        pts[mt] = (pt_pair, flo, F, off)

                    def av_denom(mt):
                        pt_pair, flo, F, off = pts.pop(mt)
                        first, last = (mt == 0), (mt == M - 1)
                        for i in range(2):
                            pa = paA if i == 0 else paB
                            pt_ = pt_pair[i]
                            nc.tensor.matmul(
                                pa[0:64, flo:CH], v3[mt][:, 0:64],
                                pt_[:, 0:F],
                                start=first, stop=last, tile_position=(0, 0))
                            nc.tensor.matmul(
                                pa[64:128, flo:CH], v3[mt][:, 64:128],
                                pt_[:, off:off + F],
                                start=first, stop=last, tile_position=(0, 64))
                        for pos, src in ((0, pt_pair[0][:, 0:F]),
                                         (32, pt_pair[0][:, off:off + F]),
                                         (64, pt_pair[1][:, 0:F]),
                                         (96, pt_pair[1][:, off:off + F])):
                            nc.tensor.matmul(
                                dn[pos:pos + 1, flo:CH], ones1[:], src,
                                start=first, stop=last,
                                tile_position=(0, pos))

                    for mt in range(M):
                        scores_exp(mt)
                        if mt > 0:
                            av_denom(mt - 1)
                        flo = max(0, (mt - 4 * ci) * 128)
                        F = CH - flo
                        slack = 2 * (2 * F + 352) / 1.2 - (5 * F / 2.4 + 400)
                        drain(max(0, slack))
                    av_denom(M - 1)

                    # Evacuate pa raw (frees the accumulator banks for the
                    # next group immediately); normalize off-critical-path.
                    aots = []
                    for i in range(2):
                        aot = wpool.tile([128, CH], bf16, tag=f"aot{i}",
                                         name="aot")
                        nc.vector.tensor_copy(aot[:], (paA if i == 0 else paB)[:])
                        aots.append(aot)
                    d4r = wpool.tile([128, CH], f32, tag="d4r", name="d4r")
                    nc.vector.reciprocal_approx_fast(d4r[0:97, :], dn[0:97, :])
                    # broadcast 1/denom rows to 128 partitions via K=1 PE
                    # matmuls into the (now free) score banks
                    for i, a in enumerate((a0, a1)):
                        rb = ps_s.tile([128, 1024], f32, tag=f"s{i}",
                                       name="rb")
                        r0, r1 = 64 * i, 64 * i + 32
                        nc.tensor.matmul(rb[0:64, 0:CH],
                                         onesb[r0:r0 + 1, :],
                                         d4r[r0:r0 + 1, :], start=True,
                                         stop=True, tile_position=(r0, 0))
                        nc.tensor.matmul(rb[64:128, CH:2 * CH],
                                         onesb[r1:r1 + 1, :],
                                         d4r[r1:r1 + 1, :], start=True,
                                         stop=True, tile_position=(r1, 64))
                        rbr = wpool.tile([128, CH], f32, tag=f"rbr{i}",
                                         name="rbr")
                        nc.scalar.copy(rbr[0:64, :], rb[0:64, 0:CH])
                        nc.scalar.copy(rbr[64:128, :], rb[64:128, CH:2 * CH])
                        an = wpool.tile([128, CH], bf16, tag=f"an{a}",
                                        name=f"an{a}")
                        nc.vector.tensor_mul(an[:], aots[i][:], rbr[:])
                        an_tiles.append(an)
                return an_tiles

            # ---- main schedule ------------------------------------------
            for fn, _ in proj_thunks(0):
                fn()
            an_by_ci = {}
            for ci in range(NCHUNK):
                if ci < NCHUNK - 1:
                    fillers.extend(proj_thunks(ci + 1))
                if ci >= 1:
                    fillers.extend(oproj_thunks(ci - 1, an_by_ci[ci - 1]))
                an_by_ci[ci] = attn_chunk(ci)
                drain_all()
            for fn, _ in oproj_thunks(NCHUNK - 1, an_by_ci[NCHUNK - 1],
                                      tail=True):
                fn()
    nc.compile()
    return nc


def _prep_in_maps(x, Wq, Wk, Wv, Wo):
    import jax.numpy as jnp

    def to_bf16(a):
        return np.asarray(jnp.asarray(np.asarray(a), dtype=jnp.bfloat16))

    i = np.arange(128)[:, None]
    j = np.arange(128)[None, :]
    msk = (i <= j).astype(np.float32)

    in_maps = []
    for c in range(N_CORES):
        b, g = c // 4, c % 4
        qh = [8 * g + a for a in range(8)]
        wq_cols = []
        for a in range(4):
            wq_cols.append(np.arange(qh[a] * HD, (qh[a] + 1) * HD))
            wq_cols.append(np.arange(qh[a + 4] * HD, (qh[a + 4] + 1) * HD))
        wq_r = np.asarray(Wq)[:, np.concatenate(wq_cols)]
        wo_r = np.asarray(Wo)[np.concatenate(wq_cols), :]
        wk_s = np.asarray(Wk)[:, 2 * g * HD: (2 * g + 2) * HD]
        wv_s = np.asarray(Wv)[:, 2 * g * HD: (2 * g + 2) * HD]
        in_maps.append({
            "xT": to_bf16(np.asarray(x)[b].T),
            "wq": to_bf16(wq_r),
            "wk": to_bf16(wk_s),
            "wv": to_bf16(wv_s),
            "wo": to_bf16(wo_r),
            "msk": to_bf16(msk),
        })
    return in_maps


def kernel(x, Wq, Wk, Wv, Wo, trace=False):
    if "nc" not in _CACHE:
        _CACHE["nc"] = _build()
    nc = _CACHE["nc"]
    in_maps = _prep_in_maps(x, Wq, Wk, Wv, Wo)
    res = bass_utils.run_bass_kernel_spmd(
        nc, in_maps, core_ids=list(range(N_CORES)), trace=trace)
    _CACHE["last_result"] = res
    out = np.zeros((B, N, D), np.float32)
    for c in range(N_CORES):
        out[c // 4] += np.asarray(res.results[c]["part"], dtype=np.float32)
    return out


# revision 27
# speedup vs baseline: 1.1786x; 1.1786x over previous
"""GroupedQueryAttention forward on 8 Trainium2 NeuronCores (Bass/Tile), v2.

Sharding (per spec hint): data-parallel over batch (B=2) x tensor-parallel
over KV-head groups (4 groups of 2 KV heads + their 8 query heads each).
Core c -> (batch b = c // 4, group g = c % 4).

v2 redesign vs baseline (573us):
  - scores row-tiled 2x: the two K=64 (per-kv-head) score matmuls run
    concurrently in disjoint PE row groups (tile_position (0,0)/(64,0)).
  - attn@V col-tiled 2x: the two M=64 AV matmuls run concurrently in
    disjoint PE col groups ((0,0)/(0,64)), both kv heads in one PSUM bank.
  - softmax denominators via 4 concurrent M=1 matmuls (ones stationary)
    col-tiled into one PSUM bank; reciprocal via DVE approx; broadcast to
    128 partitions via stride-0-source DMA; normalize fused into the PSUM
    evacuation tensor_tensor.
  - causal triangle trimming: diagonal-block matmuls/exps only cover the
    columns right of the diagonal (query >= key block start).
  - exp batched to FD=2*F per ACTIVATE from bf16 score PSUM banks.
  - mt-outer loop reuses kT/V stationaries across the 2 head pairs of a
    group; V transposed via DMA xbar instead of the PE.
  - software-pipelined emission: projection and o_proj matmul groups are
    interleaved into the attention loop as PE fillers so the tensor engine
    never idles while the scalar engine chews exps (keeps HAM at K=8/8).
  - o_proj partials written as bf16 (halves output DMA); host sums fp32.

All device compute bf16 with fp32 PSUM accumulation (bf16 PSUM for raw
scores only, pre-exp). Host pre-casts/pre-transposes x and pre-slices and
pair-reorders the weight shards.
"""

from collections import deque

import numpy as np

import concourse.bass as bass  # noqa: F401  (import keeps engine registry warm)
import concourse.mybir as mybir
import concourse.tile as tile
from concourse import bacc, bass_utils

# Problem shape (hardcoded per contract).
B, N, D = 2, 2048, 2048
NUM_HEADS = 32
NUM_KV_HEADS = 8
HD = 64
G = NUM_HEADS // NUM_KV_HEADS  # 4
N_CORES = 8
NT = D // 128                  # 16 contraction tiles
NCHUNK = 4                     # token chunks of 512
CH = 512

_CACHE = {}


def _build():
    nc = bacc.Bacc("TRN2", target_bir_lowering=False, debug=False,
                   num_devices=N_CORES)
    f32, bf16 = mybir.dt.float32, mybir.dt.bfloat16
    Exp = mybir.ActivationFunctionType.Exp

    xT = nc.dram_tensor("xT", [D, N], bf16, kind="ExternalInput")
    wq = nc.dram_tensor("wq", [D, 512], bf16, kind="ExternalInput")
    wk = nc.dram_tensor("wk", [D, 128], bf16, kind="ExternalInput")
    wv = nc.dram_tensor("wv", [D, 128], bf16, kind="ExternalInput")
    wo = nc.dram_tensor("wo", [512, D], bf16, kind="ExternalInput")
    msk = nc.dram_tensor("msk", [128, 128], bf16, kind="ExternalInput")
    part = nc.dram_tensor("part", [N, D], bf16, kind="ExternalOutput")

    with tile.TileContext(nc) as tc:
        with (
            tc.tile_pool(name="const", bufs=1) as cpool,
            tc.tile_pool(name="proj", bufs=1) as kpool,
            tc.tile_pool(name="work", bufs=2) as wpool,
            tc.tile_pool(name="ps_s", bufs=1, space="PSUM") as ps_s,
            tc.tile_pool(name="ps_pa", bufs=1, space="PSUM") as ps_pa,
            tc.tile_pool(name="ps_d", bufs=1, space="PSUM") as ps_d,
            tc.tile_pool(name="ps_proj", bufs=1, space="PSUM") as ps_proj,
        ):
            # ---- inputs / constants -------------------------------------
            wk_t = cpool.tile([128, NT * 128], bf16, tag="wk")
            nc.sync.dma_start(
                wk_t[:].rearrange("p (t o) -> p t o", t=NT),
                wk.ap().rearrange("(t p) o -> p t o", p=128))
            wv_t = cpool.tile([128, NT * 128], bf16, tag="wv")
            nc.sync.dma_start(
                wv_t[:].rearrange("p (t o) -> p t o", t=NT),
                wv.ap().rearrange("(t p) o -> p t o", p=128))
            # wq/wo/msk ride the scalar-issued DMA queue so they load in
            # parallel with the sync-queue xt chunks.
            wq_t = cpool.tile([128, NT * 512], bf16, tag="wq")
            nc.scalar.dma_start(
                wq_t[:].rearrange("p (t o) -> p t o", t=NT),
                wq.ap().rearrange("(t p) o -> p t o", p=128))
            wo_t = cpool.tile([128, 4 * D], bf16, tag="wo")
            nc.gpsimd.dma_start(
                wo_t[:].rearrange("p (t o) -> p t o", t=4),
                wo.ap().rearrange("(t p) o -> p t o", p=128))
            msk_t = cpool.tile([128, 128], bf16, tag="msk")
            nc.gpsimd.dma_start(msk_t[:], msk.ap()[:])
            # x^T loaded in 4 t-range chunks with full 4KB row descriptors;
            # chunk 0 rides the sync queue alone so the first projection
            # chain can start ~7us in, later chunks prefetch via swdge.
            xtc = []
            for q in range(4):
                xq = cpool.tile([128, 4 * N], bf16, tag=f"xtc{q}")
                nc.sync.dma_start(
                    xq[:].rearrange("p (t n) -> p t n", t=4),
                    xT.ap().rearrange("(t p) n -> p t n", p=128)
                    [:, 4 * q:4 * q + 4, :])
                xtc.append(xq)

            def xt_rhs(t, j):
                return xtc[t // 4][:, (t % 4) * N + j * CH:
                                   (t % 4) * N + (j + 1) * CH]
            ones1 = cpool.tile([128, 1], bf16, tag="ones1")
            nc.vector.memset(ones1[:], 1.0)
            onesb = cpool.tile([128, 64], f32, tag="onesb")
            nc.vector.memset(onesb[:], 1.0)
            # pre-warm the exp activation table while the DMAs run
            scr = cpool.tile([1, 8], f32, tag="scr")
            nc.vector.memset(scr[:], 0.0)
            nc.scalar.activation(scr[0:1, :], scr[0:1, :], Exp)

            # persistent projection outputs
            kt = [kpool.tile([128, CH], bf16, tag=f"kt{j}", name=f"kt{j}")
                  for j in range(NCHUNK)]
            v3 = [kpool.tile([128, 128], bf16, tag=f"v3_{m}", name=f"v3_{m}")
                  for m in range(N // 128)]
            qt = [[kpool.tile([128, CH], bf16, tag=f"qt{a}_{j}",
                              name=f"qt{a}_{j}")
                   for j in range(NCHUNK)] for a in range(4)]

            # ---- filler machinery ---------------------------------------
            fillers = deque()  # (emit_fn, est_pe_ns)

            def drain(budget_ns):
                spent = 0
                while fillers and spent < budget_ns:
                    fn, est = fillers.popleft()
                    fn()
                    spent += est

            def drain_all():
                while fillers:
                    fn, _ = fillers.popleft()
                    fn()

            # ---- projection thunks --------------------------------------
            def proj_chain(dst_evac, lhsT_col, j, n_chunks=4):
                """Returns thunk fns for one 16-deep contraction chain.

                lhsT_col(t) -> AP for the stationary tile;
                dst_evac(ps) emits the evacuation."""
                ps_box = {}

                def pair(q):
                    def emit():
                        if q == 0:
                            ps_box["ps"] = ps_proj.tile(
                                [128, CH], f32, tag="proj", name="pps")
                        ps = ps_box["ps"]
                        for t in range(2 * q, 2 * q + 2):
                            nc.tensor.matmul(
                                ps[:], lhsT_col(t), xt_rhs(t, j),
                                start=(t == 0), stop=(t == NT - 1))
                        if q == 7:
                            dst_evac(ps)
                    return emit
                return [(pair(q), 600) for q in range(8)]

            def proj_thunks(j):
                th = []

                def kev(ps):
                    nc.vector.tensor_copy(kt[j][:], ps[:])
                th += proj_chain(kev, lambda t: wk_t[:, t * 128:(t + 1) * 128], j)

                def vev(ps):
                    vt_s = wpool.tile([128, CH], bf16, tag="vt", name="vt_s")
                    nc.vector.tensor_copy(vt_s[:], ps[:])
                    for s in range(4):
                        nc.sync.dma_start_transpose(
                            v3[4 * j + s][:], vt_s[:, s * 128:(s + 1) * 128])
                th += proj_chain(vev, lambda t: wv_t[:, t * 128:(t + 1) * 128], j)
                for a in range(4):
                    def qev(ps, a=a):
                        nc.vector.tensor_copy(qt[a][j][:], ps[:])
                    th += proj_chain(
                        qev,
                        lambda t, a=a: wq_t[:, t * 512 + a * 128:
                                            t * 512 + (a + 1) * 128], j)
                return th

            # ---- o_proj thunks ------------------------------------------
            def oproj_thunks(ci, an_tiles, tail=False):
                n0 = ci * CH
                th = []
                for nt_ in range(4):
                    for dc in range(4):
                        def emit(nt_=nt_, dc=dc):
                            # in the tail (post-attention) phase the score
                            # banks are free: rotate over them too so the
                            # PE never waits on a single bank's evacuation
                            if tail and (nt_ * 4 + dc) % 2:
                                po = ps_s.tile([128, 1024], f32,
                                               tag=f"s{(nt_ * 4 + dc) // 2 % 2}",
                                               name="po2")[:, 0:CH]
                            else:
                                po = ps_proj.tile([128, CH], f32, tag="proj",
                                                  name="po")
                            for a in range(4):
                                nc.tensor.matmul(
                                    po[:],
                                    an_tiles[a][:, nt_ * 128:(nt_ + 1) * 128],
                                    wo_t[:, a * D + dc * CH:
                                         a * D + (dc + 1) * CH],
                                    start=(a == 0), stop=(a == 3))
                            st = wpool.tile([128, CH], bf16, tag="st",
                                            bufs=3, name="st")
                            if (nt_ + dc) % 2 == 0:
                                nc.scalar.copy(st[:], po[:])
                            else:
                                nc.vector.tensor_copy(st[:], po[:])
                            nc.gpsimd.dma_start(
                                part.ap()[n0 + nt_ * 128:n0 + (nt_ + 1) * 128,
                                          dc * CH:(dc + 1) * CH],
                                st[:])
                        th.append((emit, 1400))
                return th

            # ---- attention ----------------------------------------------
            def attn_chunk(ci):
                n0 = ci * CH
                M = 4 * ci + 4
                an_tiles = []
                for g in range(2):
                    a0, a1 = 2 * g, 2 * g + 1
                    paA = ps_pa.tile([128, CH], f32, tag="paA", name="paA")
                    paB = ps_pa.tile([128, CH], f32, tag="paB", name="paB")
                    dn = ps_d.tile([128, CH], f32, tag="dn", name="dn")
                    pts = {}

                    def scores_exp(mt):
                        # score layout per head pair a: kv0 block at cols
                        # [0:F], kv1 block at cols [512:512+F] (bank 2);
                        # one FD=512+F exp covers both (cols [F:512] are
                        # junk for diagonal tiles and never read).
                        jmt, cmt = mt // 4, mt % 4
                        flo = max(0, (mt - 4 * ci) * 128)
                        F = CH - flo
                        # kv1 block always in bank 1: the two row-tiled
                        # score matmuls run concurrently and must not
                        # target the same PSUM bank (cols [F:CH] junk)
                        off = CH
                        sss, pt_pair = [], []
                        for i, a in enumerate((a0, a1)):
                            ss = ps_s.tile([128, 1024], f32, tag=f"s{i}",
                                           name="ss")
                            nc.tensor.matmul(
                                ss[:, 0:F],
                                kt[jmt][0:64, cmt * 128:(cmt + 1) * 128],
                                qt[a][ci][0:64, flo:CH],
                                start=True, stop=True, tile_position=(0, 0))
                            nc.tensor.matmul(
                                ss[:, off:off + F],
                                kt[jmt][64:128, cmt * 128:(cmt + 1) * 128],
                                qt[a][ci][64:128, flo:CH],
                                start=True, stop=True, tile_position=(64, 0))
                            sss.append(ss)
                        for i in range(2):
                            pt_ = wpool.tile([128, 1024], bf16, tag=f"pt{i}",
                                             name="pt")
                            nc.scalar.activation(pt_[:, 0:off + F],
                                                 sss[i][:, 0:off + F],
                                                 Exp, scale=0.125)
                            if mt >= 4 * ci:  # diagonal block: causal mask
                                nc.vector.tensor_mul(
                                    pt_[:, 0:128], pt_[:, 0:128], msk_t[:])
                                nc.vector.tensor_mul(
                                    pt_[:, off:off + 128],
                                    pt_[:, off:off + 128], msk_t[:])
                            pt_pair.append(pt_)
                        pts[mt] = (pt_pair, flo, F, off)

                    def av_denom(mt):
                        pt_pair, flo, F, off = pts.pop(mt)
                        first, last = (mt == 0), (mt == M - 1)
                        for i in range(2):
                            pa = paA if i == 0 else paB
                            pt_ = pt_pair[i]
                            nc.tensor.matmul(
                                pa[0:64, flo:CH], v3[mt][:, 0:64],
                                pt_[:, 0:F],
                                start=first, stop=last, tile_position=(0, 0))
                            nc.tensor.matmul(
                                pa[64:128, flo:CH], v3[mt][:, 64:128],
                                pt_[:, off:off + F],
                                start=first, stop=last, tile_position=(0, 64))
                        for pos, src in ((0, pt_pair[0][:, 0:F]),
                                         (32, pt_pair[0][:, off:off + F]),
                                         (64, pt_pair[1][:, 0:F]),
                                         (96, pt_pair[1][:, off:off + F])):
                            nc.tensor.matmul(
                                dn[pos:pos + 1, flo:CH], ones1[:], src,
                                start=first, stop=last,
                                tile_position=(0, pos))

                    for mt in range(M):
                        scores_exp(mt)
                        if mt > 0:
                            av_denom(mt - 1)
                        flo = max(0, (mt - 4 * ci) * 128)
                        F = CH - flo
                        slack = 2 * (2 * F + 352) / 1.2 - (5 * F / 2.4 + 400)
                        drain(max(0, slack))
                    av_denom(M - 1)

                    # Evacuate pa raw (frees the accumulator banks for the
                    # next group immediately); normalize off-critical-path.
                    aots = []
                    for i in range(2):
                        aot = wpool.tile([128, CH], bf16, tag=f"aot{i}",
                                         name="aot")
                        nc.vector.tensor_copy(aot[:], (paA if i == 0 else paB)[:])
                        aots.append(aot)
                    d4r = wpool.tile([128, CH], f32, tag="d4r", name="d4r")
                    nc.vector.reciprocal_approx_fast(d4r[0:97, :], dn[0:97, :])
                    # broadcast 1/denom rows to 128 partitions via K=1 PE
                    # matmuls into the (now free) score banks
                    for i, a in enumerate((a0, a1)):
                        rb = ps_s.tile([128, 1024], f32, tag=f"s{i}",
                                       name="rb")
                        r0, r1 = 64 * i, 64 * i + 32
                        nc.tensor.matmul(rb[0:64, 0:CH],
                                         onesb[r0:r0 + 1, :],
                                         d4r[r0:r0 + 1, :], start=True,
                                         stop=True, tile_position=(r0, 0))
                        nc.tensor.matmul(rb[64:128, CH:2 * CH],
                                         onesb[r1:r1 + 1, :],
                                         d4r[r1:r1 + 1, :], start=True,
                                         stop=True, tile_position=(r1, 64))
                        rbr = wpool.tile([128, CH], f32, tag=f"rbr{i}",
                                         name="rbr")
                        nc.scalar.copy(rbr[0:64, :], rb[0:64, 0:CH])
                        nc.scalar.copy(rbr[64:128, :], rb[64:128, CH:2 * CH])
                        an = wpool.tile([128, CH], bf16, tag=f"an{a}",
                                        name=f"an{a}")
                        nc.vector.tensor_mul(an[:], aots[i][:], rbr[:])
                        an_tiles.append(an)
                return an_tiles

            # ---- main schedule ------------------------------------------
            for fn, _ in proj_thunks(0):
                fn()
            an_by_ci = {}
            for ci in range(NCHUNK):
                if ci < NCHUNK - 1:
                    fillers.extend(proj_thunks(ci + 1))
                if ci >= 1:
                    fillers.extend(oproj_thunks(ci - 1, an_by_ci[ci - 1]))
                an_by_ci[ci] = attn_chunk(ci)
                drain_all()
            for fn, _ in oproj_thunks(NCHUNK - 1, an_by_ci[NCHUNK - 1],
                                      tail=True):
                fn()
    nc.compile()
    return nc


def _prep_in_maps(x, Wq, Wk, Wv, Wo):
    import jax.numpy as jnp

    def to_bf16(a):
        return np.asarray(jnp.asarray(np.asarray(a), dtype=jnp.bfloat16))

    i = np.arange(128)[:, None]
    j = np.arange(128)[None, :]
    msk = (i <= j).astype(np.float32)

    in_maps = []
    for c in range(N_CORES):
        b, g = c // 4, c % 4
        qh = [8 * g + a for a in range(8)]
        wq_cols = []
        for a in range(4):
            wq_cols.append(np.arange(qh[a] * HD, (qh[a] + 1) * HD))
            wq_cols.append(np.arange(qh[a + 4] * HD, (qh[a + 4] + 1) * HD))
        wq_r = np.asarray(Wq)[:, np.concatenate(wq_cols)]
        wo_r = np.asarray(Wo)[np.concatenate(wq_cols), :]
        wk_s = np.asarray(Wk)[:, 2 * g * HD: (2 * g + 2) * HD]
        wv_s = np.asarray(Wv)[:, 2 * g * HD: (2 * g + 2) * HD]
        in_maps.append({
            "xT": to_bf16(np.asarray(x)[b].T),
            "wq": to_bf16(wq_r),
            "wk": to_bf16(wk_s),
            "wv": to_bf16(wv_s),
            "wo": to_bf16(wo_r),
            "msk": to_bf16(msk),
        })
    return in_maps


def kernel(x, Wq, Wk, Wv, Wo, trace=False):
    if "nc" not in _CACHE:
        _CACHE["nc"] = _build()
    nc = _CACHE["nc"]
    in_maps = _prep_in_maps(x, Wq, Wk, Wv, Wo)
    res = bass_utils.run_bass_kernel_spmd(
        nc, in_maps, core_ids=list(range(N_CORES)), trace=trace)
    _CACHE["last_result"] = res
    out = np.zeros((B, N, D), np.float32)
    for c in range(N_CORES):
        out[c // 4] += np.asarray(res.results[c]["part"], dtype=np.float32)
    return out


# revision 29
# speedup vs baseline: 1.1802x; 1.0014x over previous
"""GroupedQueryAttention forward on 8 Trainium2 NeuronCores (Bass/Tile), v2.

Sharding (per spec hint): data-parallel over batch (B=2) x tensor-parallel
over KV-head groups (4 groups of 2 KV heads + their 8 query heads each).
Core c -> (batch b = c // 4, group g = c % 4).

v2 redesign vs baseline (573us):
  - scores row-tiled 2x: the two K=64 (per-kv-head) score matmuls run
    concurrently in disjoint PE row groups (tile_position (0,0)/(64,0)).
  - attn@V col-tiled 2x: the two M=64 AV matmuls run concurrently in
    disjoint PE col groups ((0,0)/(0,64)), both kv heads in one PSUM bank.
  - softmax denominators via 4 concurrent M=1 matmuls (ones stationary)
    col-tiled into one PSUM bank; reciprocal via DVE approx; broadcast to
    128 partitions via stride-0-source DMA; normalize fused into the PSUM
    evacuation tensor_tensor.
  - causal triangle trimming: diagonal-block matmuls/exps only cover the
    columns right of the diagonal (query >= key block start).
  - exp batched to FD=2*F per ACTIVATE from bf16 score PSUM banks.
  - mt-outer loop reuses kT/V stationaries across the 2 head pairs of a
    group; V transposed via DMA xbar instead of the PE.
  - software-pipelined emission: projection and o_proj matmul groups are
    interleaved into the attention loop as PE fillers so the tensor engine
    never idles while the scalar engine chews exps (keeps HAM at K=8/8).
  - o_proj partials written as bf16 (halves output DMA); host sums fp32.

All device compute bf16 with fp32 PSUM accumulation (bf16 PSUM for raw
scores only, pre-exp). Host pre-casts/pre-transposes x and pre-slices and
pair-reorders the weight shards.
"""

from collections import deque

import numpy as np

import concourse.bass as bass  # noqa: F401  (import keeps engine registry warm)
import concourse.mybir as mybir
import concourse.tile as tile
from concourse import bacc, bass_utils

# Problem shape (hardcoded per contract).
B, N, D = 2, 2048, 2048
NUM_HEADS = 32
NUM_KV_HEADS = 8
HD = 64
G = NUM_HEADS // NUM_KV_HEADS  # 4
N_CORES = 8
NT = D // 128                  # 16 contraction tiles
NCHUNK = 4                     # token chunks of 512
CH = 512

_CACHE = {}


def _build():
    nc = bacc.Bacc("TRN2", target_bir_lowering=False, debug=False,
                   num_devices=N_CORES)
    f32, bf16 = mybir.dt.float32, mybir.dt.bfloat16
    Exp = mybir.ActivationFunctionType.Exp

    xT = nc.dram_tensor("xT", [D, N], bf16, kind="ExternalInput")
    wq = nc.dram_tensor("wq", [D, 512], bf16, kind="ExternalInput")
    wk = nc.dram_tensor("wk", [D, 128], bf16, kind="ExternalInput")
    wv = nc.dram_tensor("wv", [D, 128], bf16, kind="ExternalInput")
    wo = nc.dram_tensor("wo", [512, D], bf16, kind="ExternalInput")
    msk = nc.dram_tensor("msk", [128, 128], bf16, kind="ExternalInput")
    part = nc.dram_tensor("part", [N, D], bf16, kind="ExternalOutput")

    with tile.TileContext(nc) as tc:
        with (
            tc.tile_pool(name="const", bufs=1) as cpool,
            tc.tile_pool(name="proj", bufs=1) as kpool,
            tc.tile_pool(name="work", bufs=2) as wpool,
            tc.tile_pool(name="ps_s", bufs=1, space="PSUM") as ps_s,
            tc.tile_pool(name="ps_pa", bufs=1, space="PSUM") as ps_pa,
            tc.tile_pool(name="ps_d", bufs=1, space="PSUM") as ps_d,
            tc.tile_pool(name="ps_proj", bufs=1, space="PSUM") as ps_proj,
        ):
            # ---- inputs / constants -------------------------------------
            wk_t = cpool.tile([128, NT * 128], bf16, tag="wk")
            nc.sync.dma_start(
                wk_t[:].rearrange("p (t o) -> p t o", t=NT),
                wk.ap().rearrange("(t p) o -> p t o", p=128))
            wv_t = cpool.tile([128, NT * 128], bf16, tag="wv")
            nc.sync.dma_start(
                wv_t[:].rearrange("p (t o) -> p t o", t=NT),
                wv.ap().rearrange("(t p) o -> p t o", p=128))
            # wq/wo/msk ride the scalar-issued DMA queue so they load in
            # parallel with the sync-queue xt chunks.
            wq_t = cpool.tile([128, NT * 512], bf16, tag="wq")
            nc.scalar.dma_start(
                wq_t[:].rearrange("p (t o) -> p t o", t=NT),
                wq.ap().rearrange("(t p) o -> p t o", p=128))
            wo_t = cpool.tile([128, 4 * D], bf16, tag="wo")
            nc.gpsimd.dma_start(
                wo_t[:].rearrange("p (t o) -> p t o", t=4),
                wo.ap().rearrange("(t p) o -> p t o", p=128))
            msk_t = cpool.tile([128, 128], bf16, tag="msk")
            nc.gpsimd.dma_start(msk_t[:], msk.ap()[:])
            # x^T loaded in 4 t-range chunks with full 4KB row descriptors;
            # chunk 0 rides the sync queue alone so the first projection
            # chain can start ~7us in, later chunks prefetch via swdge.
            xtc = []
            for q in range(8):
                xq = cpool.tile([128, 2 * N], bf16, tag=f"xtc{q}")
                nc.sync.dma_start(
                    xq[:].rearrange("p (t n) -> p t n", t=2),
                    xT.ap().rearrange("(t p) n -> p t n", p=128)
                    [:, 2 * q:2 * q + 2, :])
                xtc.append(xq)

            def xt_rhs(t, j):
                return xtc[t // 2][:, (t % 2) * N + j * CH:
                                   (t % 2) * N + (j + 1) * CH]
            ones1 = cpool.tile([128, 1], bf16, tag="ones1")
            nc.vector.memset(ones1[:], 1.0)
            onesb = cpool.tile([128, 64], f32, tag="onesb")
            nc.vector.memset(onesb[:], 1.0)
            # pre-warm the exp activation table while the DMAs run
            scr = cpool.tile([1, 8], f32, tag="scr")
            nc.vector.memset(scr[:], 0.0)
            nc.scalar.activation(scr[0:1, :], scr[0:1, :], Exp)

            # persistent projection outputs
            kt = [kpool.tile([128, CH], bf16, tag=f"kt{j}", name=f"kt{j}")
                  for j in range(NCHUNK)]
            v3 = [kpool.tile([128, 128], bf16, tag=f"v3_{m}", name=f"v3_{m}")
                  for m in range(N // 128)]
            qt = [[kpool.tile([128, CH], bf16, tag=f"qt{a}_{j}",
                              name=f"qt{a}_{j}")
                   for j in range(NCHUNK)] for a in range(4)]

            # ---- filler machinery ---------------------------------------
            fillers = deque()  # (emit_fn, est_pe_ns)

            def drain(budget_ns):
                spent = 0
                while fillers and spent < budget_ns:
                    fn, est = fillers.popleft()
                    fn()
                    spent += est

            def drain_all():
                while fillers:
                    fn, _ = fillers.popleft()
                    fn()

            # ---- projection thunks --------------------------------------
            def proj_chain(dst_evac, lhsT_col, j, n_chunks=4):
                """Returns thunk fns for one 16-deep contraction chain.

                lhsT_col(t) -> AP for the stationary tile;
                dst_evac(ps) emits the evacuation."""
                ps_box = {}

                def pair(q):
                    def emit():
                        if q == 0:
                            ps_box["ps"] = ps_proj.tile(
                                [128, CH], f32, tag="proj", name="pps")
                        ps = ps_box["ps"]
                        for t in range(2 * q, 2 * q + 2):
                            nc.tensor.matmul(
                                ps[:], lhsT_col(t), xt_rhs(t, j),
                                start=(t == 0), stop=(t == NT - 1))
                        if q == 7:
                            dst_evac(ps)
                    return emit
                return [(pair(q), 600) for q in range(8)]

            def proj_thunks(j):
                th = []

                def kev(ps):
                    nc.vector.tensor_copy(kt[j][:], ps[:])
                th += proj_chain(kev, lambda t: wk_t[:, t * 128:(t + 1) * 128], j)

                def vev(ps):
                    vt_s = wpool.tile([128, CH], bf16, tag="vt", name="vt_s")
                    nc.vector.tensor_copy(vt_s[:], ps[:])
                    for s in range(4):
                        nc.sync.dma_start_transpose(
                            v3[4 * j + s][:], vt_s[:, s * 128:(s + 1) * 128])
                th += proj_chain(vev, lambda t: wv_t[:, t * 128:(t + 1) * 128], j)
                for a in range(4):
                    def qev(ps, a=a):
                        nc.vector.tensor_copy(qt[a][j][:], ps[:])
                    th += proj_chain(
                        qev,
                        lambda t, a=a: wq_t[:, t * 512 + a * 128:
                                            t * 512 + (a + 1) * 128], j)
                return th

            # ---- o_proj thunks ------------------------------------------
            def oproj_thunks(ci, an_tiles, tail=False):
                n0 = ci * CH
                th = []
                for nt_ in range(4):
                    for dc in range(4):
                        def emit(nt_=nt_, dc=dc):
                            # in the tail (post-attention) phase the score
                            # banks are free: rotate over them too so the
                            # PE never waits on a single bank's evacuation
                            if tail and (nt_ * 4 + dc) % 2:
                                po = ps_s.tile([128, 1024], f32,
                                               tag=f"s{(nt_ * 4 + dc) // 2 % 2}",
                                               name="po2")[:, 0:CH]
                            else:
                                po = ps_proj.tile([128, CH], f32, tag="proj",
                                                  name="po")
                            for a in range(4):
                                nc.tensor.matmul(
                                    po[:],
                                    an_tiles[a][:, nt_ * 128:(nt_ + 1) * 128],
                                    wo_t[:, a * D + dc * CH:
                                         a * D + (dc + 1) * CH],
                                    start=(a == 0), stop=(a == 3))
                            st = wpool.tile([128, CH], bf16, tag="st",
                                            bufs=3, name="st")
                            if (nt_ + dc) % 2 == 0:
                                nc.scalar.copy(st[:], po[:])
                            else:
                                nc.vector.tensor_copy(st[:], po[:])
                            nc.gpsimd.dma_start(
                                part.ap()[n0 + nt_ * 128:n0 + (nt_ + 1) * 128,
                                          dc * CH:(dc + 1) * CH],
                                st[:])
                        th.append((emit, 1400))
                return th

            # ---- attention ----------------------------------------------
            def attn_chunk(ci):
                n0 = ci * CH
                M = 4 * ci + 4
                an_tiles = []
                for g in range(2):
                    a0, a1 = 2 * g, 2 * g + 1
                    paA = ps_pa.tile([128, CH], f32, tag="paA", name="paA")
                    paB = ps_pa.tile([128, CH], f32, tag="paB", name="paB")
                    dn = ps_d.tile([128, CH], f32, tag="dn", name="dn")
                    pts = {}

                    def scores_exp(mt):
                        # score layout per head pair a: kv0 block at cols
                        # [0:F], kv1 block at cols [512:512+F] (bank 2);
                        # one FD=512+F exp covers both (cols [F:512] are
                        # junk for diagonal tiles and never read).
                        jmt, cmt = mt // 4, mt % 4
                        flo = max(0, (mt - 4 * ci) * 128)
                        F = CH - flo
                        # kv1 block always in bank 1: the two row-tiled
                        # score matmuls run concurrently and must not
                        # target the same PSUM bank (cols [F:CH] junk)
                        off = CH
                        sss, pt_pair = [], []
                        for i, a in enumerate((a0, a1)):
                            ss = ps_s.tile([128, 1024], f32, tag=f"s{i}",
                                           name="ss")
                            nc.tensor.matmul(
                                ss[:, 0:F],
                                kt[jmt][0:64, cmt * 128:(cmt + 1) * 128],
                                qt[a][ci][0:64, flo:CH],
                                start=True, stop=True, tile_position=(0, 0))
                            nc.tensor.matmul(
                                ss[:, off:off + F],
                                kt[jmt][64:128, cmt * 128:(cmt + 1) * 128],
                                qt[a][ci][64:128, flo:CH],
                                start=True, stop=True, tile_position=(64, 0))
                            sss.append(ss)
                        for i in range(2):
                            pt_ = wpool.tile([128, 1024], bf16, tag=f"pt{i}",
                                             name="pt")
                            nc.scalar.activation(pt_[:, 0:off + F],
                                                 sss[i][:, 0:off + F],
                                                 Exp, scale=0.125)
                            if mt >= 4 * ci:  # diagonal block: causal mask
                                nc.vector.tensor_mul(
                                    pt_[:, 0:128], pt_[:, 0:128], msk_t[:])
                                nc.vector.tensor_mul(
                                    pt_[:, off:off + 128],
                                    pt_[:, off:off + 128], msk_t[:])
                            pt_pair.append(pt_)
                        pts[mt] = (pt_pair, flo, F, off)

                    def av_denom(mt):
                        pt_pair, flo, F, off = pts.pop(mt)
                        first, last = (mt == 0), (mt == M - 1)
                        for i in range(2):
                            pa = paA if i == 0 else paB
                            pt_ = pt_pair[i]
                            nc.tensor.matmul(
                                pa[0:64, flo:CH], v3[mt][:, 0:64],
                                pt_[:, 0:F],
                                start=first, stop=last, tile_position=(0, 0))
                            nc.tensor.matmul(
                                pa[64:128, flo:CH], v3[mt][:, 64:128],
                                pt_[:, off:off + F],
                                start=first, stop=last, tile_position=(0, 64))
                        for pos, src in ((0, pt_pair[0][:, 0:F]),
                                         (32, pt_pair[0][:, off:off + F]),
                                         (64, pt_pair[1][:, 0:F]),
                                         (96, pt_pair[1][:, off:off + F])):
                            nc.tensor.matmul(
                                dn[pos:pos + 1, flo:CH], ones1[:], src,
                                start=first, stop=last,
                                tile_position=(0, pos))

                    for mt in range(M):
                        scores_exp(mt)
                        if mt > 0:
                            av_denom(mt - 1)
                        flo = max(0, (mt - 4 * ci) * 128)
                        F = CH - flo
                        slack = 2 * (2 * F + 352) / 1.2 - (5 * F / 2.4 + 400)
                        drain(max(0, slack))
                    av_denom(M - 1)

                    # Evacuate pa raw (frees the accumulator banks for the
                    # next group immediately); normalize off-critical-path.
                    aots = []
                    for i in range(2):
                        aot = wpool.tile([128, CH], bf16, tag=f"aot{i}",
                                         name="aot")
                        nc.vector.tensor_copy(aot[:], (paA if i == 0 else paB)[:])
                        aots.append(aot)
                    d4r = wpool.tile([128, CH], f32, tag="d4r", name="d4r")
                    nc.vector.reciprocal_approx_fast(d4r[0:97, :], dn[0:97, :])
                    # broadcast 1/denom rows to 128 partitions via K=1 PE
                    # matmuls into the (now free) score banks: all four in
                    # one window (pairwise-disjoint PE quadrants)
                    rbs = [ps_s.tile([128, 1024], f32, tag=f"s{i}",
                                     name="rb") for i in range(2)]
                    for i in range(2):
                        r0, r1 = 64 * i, 64 * i + 32
                        nc.tensor.matmul(rbs[i][0:64, 0:CH],
                                         onesb[r0:r0 + 1, :],
                                         d4r[r0:r0 + 1, :], start=True,
                                         stop=True, tile_position=(r0, 0))
                        nc.tensor.matmul(rbs[i][64:128, CH:2 * CH],
                                         onesb[r1:r1 + 1, :],
                                         d4r[r1:r1 + 1, :], start=True,
                                         stop=True, tile_position=(r1, 64))
                    for i, a in enumerate((a0, a1)):
                        rbr = wpool.tile([128, CH], f32, tag=f"rbr{i}",
                                         name="rbr")
                        nc.scalar.copy(rbr[0:64, :], rbs[i][0:64, 0:CH])
                        nc.scalar.copy(rbr[64:128, :],
                                       rbs[i][64:128, CH:2 * CH])
                        an = wpool.tile([128, CH], bf16, tag=f"an{a}",
                                        name=f"an{a}")
                        nc.vector.tensor_mul(an[:], aots[i][:], rbr[:])
                        an_tiles.append(an)
                return an_tiles

            # ---- main schedule ------------------------------------------
            for fn, _ in proj_thunks(0):
                fn()
            an_by_ci = {}
            for ci in range(NCHUNK):
                if ci < NCHUNK - 1:
                    fillers.extend(proj_thunks(ci + 1))
                if ci >= 1:
                    fillers.extend(oproj_thunks(ci - 1, an_by_ci[ci - 1]))
                an_by_ci[ci] = attn_chunk(ci)
                drain_all()
            for fn, _ in oproj_thunks(NCHUNK - 1, an_by_ci[NCHUNK - 1],
                                      tail=True):
                fn()
    nc.compile()
    return nc


def _prep_in_maps(x, Wq, Wk, Wv, Wo):
    import jax.numpy as jnp

    def to_bf16(a):
        return np.asarray(jnp.asarray(np.asarray(a), dtype=jnp.bfloat16))

    i = np.arange(128)[:, None]
    j = np.arange(128)[None, :]
    msk = (i <= j).astype(np.float32)

    in_maps = []
    for c in range(N_CORES):
        b, g = c // 4, c % 4
        qh = [8 * g + a for a in range(8)]
        wq_cols = []
        for a in range(4):
            wq_cols.append(np.arange(qh[a] * HD, (qh[a] + 1) * HD))
            wq_cols.append(np.arange(qh[a + 4] * HD, (qh[a + 4] + 1) * HD))
        wq_r = np.asarray(Wq)[:, np.concatenate(wq_cols)]
        wo_r = np.asarray(Wo)[np.concatenate(wq_cols), :]
        wk_s = np.asarray(Wk)[:, 2 * g * HD: (2 * g + 2) * HD]
        wv_s = np.asarray(Wv)[:, 2 * g * HD: (2 * g + 2) * HD]
        in_maps.append({
            "xT": to_bf16(np.asarray(x)[b].T),
            "wq": to_bf16(wq_r),
            "wk": to_bf16(wk_s),
            "wv": to_bf16(wv_s),
            "wo": to_bf16(wo_r),
            "msk": to_bf16(msk),
        })
    return in_maps


def kernel(x, Wq, Wk, Wv, Wo, trace=False):
    if "nc" not in _CACHE:
        _CACHE["nc"] = _build()
    nc = _CACHE["nc"]
    in_maps = _prep_in_maps(x, Wq, Wk, Wv, Wo)
    res = bass_utils.run_bass_kernel_spmd(
        nc, in_maps, core_ids=list(range(N_CORES)), trace=trace)
    _CACHE["last_result"] = res
    out = np.zeros((B, N, D), np.float32)
    for c in range(N_CORES):
        out[c // 4] += np.asarray(res.results[c]["part"], dtype=np.float32)
    return out


# revision 32
# speedup vs baseline: 1.1876x; 1.0063x over previous
"""GroupedQueryAttention forward on 8 Trainium2 NeuronCores (Bass/Tile), v2.

Sharding (per spec hint): data-parallel over batch (B=2) x tensor-parallel
over KV-head groups (4 groups of 2 KV heads + their 8 query heads each).
Core c -> (batch b = c // 4, group g = c % 4).

v2 redesign vs baseline (573us):
  - scores row-tiled 2x: the two K=64 (per-kv-head) score matmuls run
    concurrently in disjoint PE row groups (tile_position (0,0)/(64,0)).
  - attn@V col-tiled 2x: the two M=64 AV matmuls run concurrently in
    disjoint PE col groups ((0,0)/(0,64)), both kv heads in one PSUM bank.
  - softmax denominators via 4 concurrent M=1 matmuls (ones stationary)
    col-tiled into one PSUM bank; reciprocal via DVE approx; broadcast to
    128 partitions via stride-0-source DMA; normalize fused into the PSUM
    evacuation tensor_tensor.
  - causal triangle trimming: diagonal-block matmuls/exps only cover the
    columns right of the diagonal (query >= key block start).
  - exp batched to FD=2*F per ACTIVATE from bf16 score PSUM banks.
  - mt-outer loop reuses kT/V stationaries across the 2 head pairs of a
    group; V transposed via DMA xbar instead of the PE.
  - software-pipelined emission: projection and o_proj matmul groups are
    interleaved into the attention loop as PE fillers so the tensor engine
    never idles while the scalar engine chews exps (keeps HAM at K=8/8).
  - o_proj partials written as bf16 (halves output DMA); host sums fp32.

All device compute bf16 with fp32 PSUM accumulation (bf16 PSUM for raw
scores only, pre-exp). Host pre-casts/pre-transposes x and pre-slices and
pair-reorders the weight shards.
"""

from collections import deque

import numpy as np

import concourse.bass as bass  # noqa: F401  (import keeps engine registry warm)
import concourse.mybir as mybir
import concourse.tile as tile
from concourse import bacc, bass_utils

# Problem shape (hardcoded per contract).
B, N, D = 2, 2048, 2048
NUM_HEADS = 32
NUM_KV_HEADS = 8
HD = 64
G = NUM_HEADS // NUM_KV_HEADS  # 4
N_CORES = 8
NT = D // 128                  # 16 contraction tiles
NCHUNK = 4                     # token chunks of 512
CH = 512

_CACHE = {}


def _build():
    nc = bacc.Bacc("TRN2", target_bir_lowering=False, debug=False,
                   num_devices=N_CORES)
    f32, bf16 = mybir.dt.float32, mybir.dt.bfloat16
    Exp = mybir.ActivationFunctionType.Exp

    # all inputs pre-arranged host-side into the SBUF tile layout
    # (partition-major) so every load is contiguous per partition
    xT = nc.dram_tensor("xT", [128, NT * N], bf16, kind="ExternalInput")
    wq = nc.dram_tensor("wq", [128, NT * 512], bf16, kind="ExternalInput")
    wk = nc.dram_tensor("wk", [128, NT * 128], bf16, kind="ExternalInput")
    wv = nc.dram_tensor("wv", [128, NT * 128], bf16, kind="ExternalInput")
    wo = nc.dram_tensor("wo", [128, 4 * D], bf16, kind="ExternalInput")
    msk = nc.dram_tensor("msk", [128, 128], bf16, kind="ExternalInput")
    part = nc.dram_tensor("part", [N, D], bf16, kind="ExternalOutput")

    with tile.TileContext(nc) as tc:
        with (
            tc.tile_pool(name="const", bufs=1) as cpool,
            tc.tile_pool(name="proj", bufs=1) as kpool,
            tc.tile_pool(name="work", bufs=2) as wpool,
            tc.tile_pool(name="ps_s", bufs=1, space="PSUM") as ps_s,
            tc.tile_pool(name="ps_pa", bufs=1, space="PSUM") as ps_pa,
            tc.tile_pool(name="ps_d", bufs=1, space="PSUM") as ps_d,
            tc.tile_pool(name="ps_proj", bufs=1, space="PSUM") as ps_proj,
        ):
            # ---- inputs / constants -------------------------------------
            wk_t = cpool.tile([128, NT * 128], bf16, tag="wk")
            nc.sync.dma_start(wk_t[:], wk.ap()[:])
            wv_t = cpool.tile([128, NT * 128], bf16, tag="wv")
            nc.sync.dma_start(wv_t[:], wv.ap()[:])
            # wq/wo/msk ride other queues so they load in parallel with
            # the sync-queue xt chunks.
            wq_t = cpool.tile([128, NT * 512], bf16, tag="wq")
            nc.scalar.dma_start(wq_t[:], wq.ap()[:])
            wo_t = cpool.tile([128, 4 * D], bf16, tag="wo")
            nc.gpsimd.dma_start(wo_t[:], wo.ap()[:])
            msk_t = cpool.tile([128, 128], bf16, tag="msk")
            nc.gpsimd.dma_start(msk_t[:], msk.ap()[:])
            # x^T in 8 t-range chunks, fully contiguous per partition
            xtc = []
            for q in range(8):
                xq = cpool.tile([128, 2 * N], bf16, tag=f"xtc{q}")
                nc.sync.dma_start(xq[:],
                                  xT.ap()[:, 2 * q * N:(2 * q + 2) * N])
                xtc.append(xq)

            def xt_rhs(t, j):
                return xtc[t // 2][:, (t % 2) * N + j * CH:
                                   (t % 2) * N + (j + 1) * CH]
            ones1 = cpool.tile([128, 1], bf16, tag="ones1")
            nc.vector.memset(ones1[:], 1.0)
            onesb = cpool.tile([128, 64], f32, tag="onesb")
            nc.vector.memset(onesb[:], 1.0)
            # pre-warm the exp activation table while the DMAs run
            scr = cpool.tile([1, 8], f32, tag="scr")
            nc.vector.memset(scr[:], 0.0)
            nc.scalar.activation(scr[0:1, :], scr[0:1, :], Exp)

            # persistent projection outputs
            kt = [kpool.tile([128, CH], bf16, tag=f"kt{j}", name=f"kt{j}")
                  for j in range(NCHUNK)]
            v3 = [kpool.tile([128, 128], bf16, tag=f"v3_{m}", name=f"v3_{m}")
                  for m in range(N // 128)]
            qt = [[kpool.tile([128, CH], bf16, tag=f"qt{a}_{j}",
                              name=f"qt{a}_{j}")
                   for j in range(NCHUNK)] for a in range(4)]

            # ---- filler machinery ---------------------------------------
            fillers = deque()  # (emit_fn, est_pe_ns)

            def drain(budget_ns):
                spent = 0
                while fillers and spent < budget_ns:
                    fn, est = fillers.popleft()
                    fn()
                    spent += est

            def drain_all():
                while fillers:
                    fn, _ = fillers.popleft()
                    fn()

            # ---- projection thunks --------------------------------------
            def proj_chain(dst_evac, lhsT_col, j, n_chunks=4):
                """Returns thunk fns for one 16-deep contraction chain.

                lhsT_col(t) -> AP for the stationary tile;
                dst_evac(ps) emits the evacuation."""
                ps_box = {}

                def pair(q):
                    def emit():
                        if q == 0:
                            ps_box["ps"] = ps_proj.tile(
                                [128, CH], f32, tag="proj", name="pps")
                        ps = ps_box["ps"]
                        for t in range(2 * q, 2 * q + 2):
                            nc.tensor.matmul(
                                ps[:], lhsT_col(t), xt_rhs(t, j),
                                start=(t == 0), stop=(t == NT - 1))
                        if q == 7:
                            dst_evac(ps)
                    return emit
                return [(pair(q), 600) for q in range(8)]

            def proj_thunks(j):
                th = []

                def kev(ps):
                    nc.vector.tensor_copy(kt[j][:], ps[:])
                th += proj_chain(kev, lambda t: wk_t[:, t * 128:(t + 1) * 128], j)

                def vev(ps):
                    vt_s = wpool.tile([128, CH], bf16, tag="vt", name="vt_s")
                    nc.vector.tensor_copy(vt_s[:], ps[:])
                    for s in range(4):
                        nc.sync.dma_start_transpose(
                            v3[4 * j + s][:], vt_s[:, s * 128:(s + 1) * 128])
                th += proj_chain(vev, lambda t: wv_t[:, t * 128:(t + 1) * 128], j)
                for a in range(4):
                    def qev(ps, a=a):
                        nc.vector.tensor_copy(qt[a][j][:], ps[:])
                    th += proj_chain(
                        qev,
                        lambda t, a=a: wq_t[:, t * 512 + a * 128:
                                            t * 512 + (a + 1) * 128], j)
                return th

            # ---- o_proj thunks ------------------------------------------
            def oproj_thunks(ci, an_tiles, tail=False):
                n0 = ci * CH
                th = []
                for nt_ in range(4):
                    for dc in range(4):
                        def emit(nt_=nt_, dc=dc):
                            # in the tail (post-attention) phase the score
                            # banks are free: rotate over them too so the
                            # PE never waits on a single bank's evacuation
                            if tail and (nt_ * 4 + dc) % 2:
                                po = ps_s.tile([128, 1024], f32,
                                               tag=f"s{(nt_ * 4 + dc) // 2 % 2}",
                                               name="po2")[:, 0:CH]
                            else:
                                po = ps_proj.tile([128, CH], f32, tag="proj",
                                                  name="po")
                            for a in range(4):
                                nc.tensor.matmul(
                                    po[:],
                                    an_tiles[a][:, nt_ * 128:(nt_ + 1) * 128],
                                    wo_t[:, a * D + dc * CH:
                                         a * D + (dc + 1) * CH],
                                    start=(a == 0), stop=(a == 3))
                            st = wpool.tile([128, CH], bf16, tag="st",
                                            bufs=3, name="st")
                            if (nt_ + dc) % 2 == 0:
                                nc.scalar.copy(st[:], po[:])
                            else:
                                nc.vector.tensor_copy(st[:], po[:])
                            nc.gpsimd.dma_start(
                                part.ap()[n0 + nt_ * 128:n0 + (nt_ + 1) * 128,
                                          dc * CH:(dc + 1) * CH],
                                st[:])
                        th.append((emit, 1400))
                return th

            # ---- attention ----------------------------------------------
            def attn_chunk(ci):
                n0 = ci * CH
                M = 4 * ci + 4
                an_tiles = []
                for g in range(2):
                    a0, a1 = 2 * g, 2 * g + 1
                    paA = ps_pa.tile([128, CH], f32, tag="paA", name="paA")
                    paB = ps_pa.tile([128, CH], f32, tag="paB", name="paB")
                    dn = ps_d.tile([128, CH], f32, tag="dn", name="dn")
                    pts = {}

                    def scores_exp(mt):
                        # score layout per head pair a: kv0 block at cols
                        # [0:F], kv1 block at cols [512:512+F] (bank 2);
                        # one FD=512+F exp covers both (cols [F:512] are
                        # junk for diagonal tiles and never read).
                        jmt, cmt = mt // 4, mt % 4
                        flo = max(0, (mt - 4 * ci) * 128)
                        F = CH - flo
                        # kv1 block always in bank 1: the two row-tiled
                        # score matmuls run concurrently and must not
                        # target the same PSUM bank (cols [F:CH] junk)
                        off = CH
                        sss, pt_pair = [], []
                        for i, a in enumerate((a0, a1)):
                            ss = ps_s.tile([128, 1024], f32, tag=f"s{i}",
                                           name="ss")
                            nc.tensor.matmul(
                                ss[:, 0:F],
                                kt[jmt][0:64, cmt * 128:(cmt + 1) * 128],
                                qt[a][ci][0:64, flo:CH],
                                start=True, stop=True, tile_position=(0, 0))
                            nc.tensor.matmul(
                                ss[:, off:off + F],
                                kt[jmt][64:128, cmt * 128:(cmt + 1) * 128],
                                qt[a][ci][64:128, flo:CH],
                                start=True, stop=True, tile_position=(64, 0))
                            sss.append(ss)
                        for i in range(2):
                            pt_ = wpool.tile([128, 1024], bf16, tag=f"pt{i}",
                                             name="pt")
                            nc.scalar.activation(pt_[:, 0:off + F],
                                                 sss[i][:, 0:off + F],
                                                 Exp, scale=0.125)
                            if mt >= 4 * ci:  # diagonal block: causal mask
                                nc.vector.tensor_mul(
                                    pt_[:, 0:128], pt_[:, 0:128], msk_t[:])
                                nc.vector.tensor_mul(
                                    pt_[:, off:off + 128],
                                    pt_[:, off:off + 128], msk_t[:])
                            pt_pair.append(pt_)
                        pts[mt] = (pt_pair, flo, F, off)

                    def av_denom(mt):
                        pt_pair, flo, F, off = pts.pop(mt)
                        first, last = (mt == 0), (mt == M - 1)
                        for i in range(2):
                            pa = paA if i == 0 else paB
                            pt_ = pt_pair[i]
                            nc.tensor.matmul(
                                pa[0:64, flo:CH], v3[mt][:, 0:64],
                                pt_[:, 0:F],
                                start=first, stop=last, tile_position=(0, 0))
                            nc.tensor.matmul(
                                pa[64:128, flo:CH], v3[mt][:, 64:128],
                                pt_[:, off:off + F],
                                start=first, stop=last, tile_position=(0, 64))
                        for pos, src in ((0, pt_pair[0][:, 0:F]),
                                         (32, pt_pair[0][:, off:off + F]),
                                         (64, pt_pair[1][:, 0:F]),
                                         (96, pt_pair[1][:, off:off + F])):
                            nc.tensor.matmul(
                                dn[pos:pos + 1, flo:CH], ones1[:], src,
                                start=first, stop=last,
                                tile_position=(0, pos))

                    for mt in range(M):
                        scores_exp(mt)
                        if mt > 0:
                            av_denom(mt - 1)
                        flo = max(0, (mt - 4 * ci) * 128)
                        F = CH - flo
                        slack = 2 * (2 * F + 352) / 1.2 - (5 * F / 2.4 + 400)
                        drain(max(0, slack))
                    av_denom(M - 1)

                    # Evacuate pa raw (frees the accumulator banks for the
                    # next group immediately); normalize off-critical-path.
                    aots = []
                    for i in range(2):
                        aot = wpool.tile([128, CH], bf16, tag=f"aot{i}",
                                         name="aot")
                        nc.vector.tensor_copy(aot[:], (paA if i == 0 else paB)[:])
                        aots.append(aot)
                    d4r = wpool.tile([128, CH], f32, tag="d4r", name="d4r")
                    nc.vector.reciprocal_approx_fast(d4r[0:97, :], dn[0:97, :])
                    # broadcast 1/denom rows to 128 partitions via K=1 PE
                    # matmuls into the (now free) score banks: all four in
                    # one window (pairwise-disjoint PE quadrants)
                    rbs = [ps_s.tile([128, 1024], f32, tag=f"s{i}",
                                     name="rb") for i in range(2)]
                    for i in range(2):
                        r0, r1 = 64 * i, 64 * i + 32
                        nc.tensor.matmul(rbs[i][0:64, 0:CH],
                                         onesb[r0:r0 + 1, :],
                                         d4r[r0:r0 + 1, :], start=True,
                                         stop=True, tile_position=(r0, 0))
                        nc.tensor.matmul(rbs[i][64:128, CH:2 * CH],
                                         onesb[r1:r1 + 1, :],
                                         d4r[r1:r1 + 1, :], start=True,
                                         stop=True, tile_position=(r1, 64))
                    for i, a in enumerate((a0, a1)):
                        rbr = wpool.tile([128, CH], f32, tag=f"rbr{i}",
                                         name="rbr")
                        nc.scalar.copy(rbr[0:64, :], rbs[i][0:64, 0:CH])
                        nc.scalar.copy(rbr[64:128, :],
                                       rbs[i][64:128, CH:2 * CH])
                        an = wpool.tile([128, CH], bf16, tag=f"an{a}",
                                        name=f"an{a}")
                        nc.vector.tensor_mul(an[:], aots[i][:], rbr[:])
                        an_tiles.append(an)
                return an_tiles

            # ---- main schedule ------------------------------------------
            for fn, _ in proj_thunks(0):
                fn()
            an_by_ci = {}
            for ci in range(NCHUNK):
                if ci < NCHUNK - 1:
                    fillers.extend(proj_thunks(ci + 1))
                if ci >= 1:
                    fillers.extend(oproj_thunks(ci - 1, an_by_ci[ci - 1]))
                an_by_ci[ci] = attn_chunk(ci)
                drain_all()
            for fn, _ in oproj_thunks(NCHUNK - 1, an_by_ci[NCHUNK - 1],
                                      tail=True):
                fn()
    nc.compile()
    return nc


def _prep_in_maps(x, Wq, Wk, Wv, Wo):
    import jax.numpy as jnp

    def to_bf16(a):
        return np.asarray(jnp.asarray(np.asarray(a), dtype=jnp.bfloat16))

    i = np.arange(128)[:, None]
    j = np.arange(128)[None, :]
    msk = (i <= j).astype(np.float32)

    def devlay(a):
        # [K*128, O] -> [128, K*O] partition-major device layout
        k = a.shape[0] // 128
        return a.reshape(k, 128, a.shape[1]).transpose(1, 0, 2).reshape(128, -1)

    in_maps = []
    for c in range(N_CORES):
        b, g = c // 4, c % 4
        qh = [8 * g + a for a in range(8)]
        wq_cols = []
        for a in range(4):
            wq_cols.append(np.arange(qh[a] * HD, (qh[a] + 1) * HD))
            wq_cols.append(np.arange(qh[a + 4] * HD, (qh[a + 4] + 1) * HD))
        wq_r = np.asarray(Wq)[:, np.concatenate(wq_cols)]
        wo_r = np.asarray(Wo)[np.concatenate(wq_cols), :]
        wk_s = np.asarray(Wk)[:, 2 * g * HD: (2 * g + 2) * HD]
        wv_s = np.asarray(Wv)[:, 2 * g * HD: (2 * g + 2) * HD]
        in_maps.append({
            "xT": to_bf16(devlay(np.ascontiguousarray(np.asarray(x)[b].T))),
            "wq": to_bf16(devlay(wq_r)),
            "wk": to_bf16(devlay(wk_s)),
            "wv": to_bf16(devlay(wv_s)),
            "wo": to_bf16(devlay(wo_r)),
            "msk": to_bf16(msk),
        })
    return in_maps


def kernel(x, Wq, Wk, Wv, Wo, trace=False):
    if "nc" not in _CACHE:
        _CACHE["nc"] = _build()
    nc = _CACHE["nc"]
    in_maps = _prep_in_maps(x, Wq, Wk, Wv, Wo)
    res = bass_utils.run_bass_kernel_spmd(
        nc, in_maps, core_ids=list(range(N_CORES)), trace=trace)
    _CACHE["last_result"] = res
    out = np.zeros((B, N, D), np.float32)
    for c in range(N_CORES):
        out[c // 4] += np.asarray(res.results[c]["part"], dtype=np.float32)
    return out


# revision 33
# speedup vs baseline: 1.2136x; 1.0218x over previous
"""GroupedQueryAttention forward on 8 Trainium2 NeuronCores (Bass/Tile), v2.

Sharding (per spec hint): data-parallel over batch (B=2) x tensor-parallel
over KV-head groups (4 groups of 2 KV heads + their 8 query heads each).
Core c -> (batch b = c // 4, group g = c % 4).

v2 redesign vs baseline (573us):
  - scores row-tiled 2x: the two K=64 (per-kv-head) score matmuls run
    concurrently in disjoint PE row groups (tile_position (0,0)/(64,0)).
  - attn@V col-tiled 2x: the two M=64 AV matmuls run concurrently in
    disjoint PE col groups ((0,0)/(0,64)), both kv heads in one PSUM bank.
  - softmax denominators via 4 concurrent M=1 matmuls (ones stationary)
    col-tiled into one PSUM bank; reciprocal via DVE approx; broadcast to
    128 partitions via stride-0-source DMA; normalize fused into the PSUM
    evacuation tensor_tensor.
  - causal triangle trimming: diagonal-block matmuls/exps only cover the
    columns right of the diagonal (query >= key block start).
  - exp batched to FD=2*F per ACTIVATE from bf16 score PSUM banks.
  - mt-outer loop reuses kT/V stationaries across the 2 head pairs of a
    group; V transposed via DMA xbar instead of the PE.
  - software-pipelined emission: projection and o_proj matmul groups are
    interleaved into the attention loop as PE fillers so the tensor engine
    never idles while the scalar engine chews exps (keeps HAM at K=8/8).
  - o_proj partials written as bf16 (halves output DMA); host sums fp32.

All device compute bf16 with fp32 PSUM accumulation (bf16 PSUM for raw
scores only, pre-exp). Host pre-casts/pre-transposes x and pre-slices and
pair-reorders the weight shards.
"""

from collections import deque

import numpy as np

import concourse.bass as bass  # noqa: F401  (import keeps engine registry warm)
import concourse.mybir as mybir
import concourse.tile as tile
from concourse import bacc, bass_utils

# Problem shape (hardcoded per contract).
B, N, D = 2, 2048, 2048
NUM_HEADS = 32
NUM_KV_HEADS = 8
HD = 64
G = NUM_HEADS // NUM_KV_HEADS  # 4
N_CORES = 8
NT = D // 128                  # 16 contraction tiles
NCHUNK = 4                     # token chunks of 512
CH = 512

_CACHE = {}


def _build():
    nc = bacc.Bacc("TRN2", target_bir_lowering=False, debug=False,
                   num_devices=N_CORES)
    f32, bf16 = mybir.dt.float32, mybir.dt.bfloat16
    Exp = mybir.ActivationFunctionType.Exp

    # all inputs pre-arranged host-side into the SBUF tile layout
    # (partition-major) so every load is contiguous per partition
    xT = nc.dram_tensor("xT", [128, NT * N], bf16, kind="ExternalInput")
    wq = nc.dram_tensor("wq", [128, NT * 512], bf16, kind="ExternalInput")
    wk = nc.dram_tensor("wk", [128, NT * 128], bf16, kind="ExternalInput")
    wv = nc.dram_tensor("wv", [128, NT * 128], bf16, kind="ExternalInput")
    wo = nc.dram_tensor("wo", [128, 4 * D], bf16, kind="ExternalInput")
    msk = nc.dram_tensor("msk", [128, 128], bf16, kind="ExternalInput")
    part = nc.dram_tensor("part", [N, D], bf16, kind="ExternalOutput")

    with tile.TileContext(nc) as tc:
        with (
            tc.tile_pool(name="const", bufs=1) as cpool,
            tc.tile_pool(name="proj", bufs=1) as kpool,
            tc.tile_pool(name="work", bufs=2) as wpool,
            tc.tile_pool(name="ps_s", bufs=1, space="PSUM") as ps_s,
            tc.tile_pool(name="ps_pa", bufs=1, space="PSUM") as ps_pa,
            tc.tile_pool(name="ps_d", bufs=1, space="PSUM") as ps_d,
            tc.tile_pool(name="ps_proj", bufs=1, space="PSUM") as ps_proj,
        ):
            # ---- inputs / constants -------------------------------------
            wk_t = cpool.tile([128, NT * 128], bf16, tag="wk")
            nc.sync.dma_start(wk_t[:], wk.ap()[:])
            wv_t = cpool.tile([128, NT * 128], bf16, tag="wv")
            nc.sync.dma_start(wv_t[:], wv.ap()[:])
            # wq/wo/msk ride other queues so they load in parallel with
            # the sync-queue xt chunks.
            wq_t = cpool.tile([128, NT * 512], bf16, tag="wq")
            nc.scalar.dma_start(wq_t[:], wq.ap()[:])
            wo_t = cpool.tile([128, 4 * D], bf16, tag="wo")
            nc.gpsimd.dma_start(wo_t[:], wo.ap()[:])
            msk_t = cpool.tile([128, 128], bf16, tag="msk")
            nc.gpsimd.dma_start(msk_t[:], msk.ap()[:])
            # x^T in 8 t-range chunks, fully contiguous per partition
            xtc = []
            for q in range(8):
                xq = cpool.tile([128, 2 * N], bf16, tag=f"xtc{q}")
                nc.sync.dma_start(xq[:],
                                  xT.ap()[:, 2 * q * N:(2 * q + 2) * N])
                xtc.append(xq)

            def xt_rhs(t, j):
                return xtc[t // 2][:, (t % 2) * N + j * CH:
                                   (t % 2) * N + (j + 1) * CH]
            ones1 = cpool.tile([128, 1], bf16, tag="ones1")
            nc.vector.memset(ones1[:], 1.0)
            onesb = cpool.tile([128, 64], f32, tag="onesb")
            nc.vector.memset(onesb[:], 1.0)
            # pre-warm the exp activation table while the DMAs run
            scr = cpool.tile([1, 8], f32, tag="scr")
            nc.vector.memset(scr[:], 0.0)
            nc.scalar.activation(scr[0:1, :], scr[0:1, :], Exp)

            # persistent projection outputs
            kt = [kpool.tile([128, CH], bf16, tag=f"kt{j}", name=f"kt{j}")
                  for j in range(NCHUNK)]
            v3 = [kpool.tile([128, 128], bf16, tag=f"v3_{m}", name=f"v3_{m}")
                  for m in range(N // 128)]
            qt = [[kpool.tile([128, CH], bf16, tag=f"qt{a}_{j}",
                              name=f"qt{a}_{j}")
                   for j in range(NCHUNK)] for a in range(4)]

            # ---- filler machinery ---------------------------------------
            fillers = deque()  # (emit_fn, est_pe_ns)

            def drain(budget_ns):
                spent = 0
                while fillers and spent < budget_ns:
                    fn, est = fillers.popleft()
                    fn()
                    spent += est

            def drain_all():
                while fillers:
                    fn, _ = fillers.popleft()
                    fn()

            # ---- projection thunks --------------------------------------
            def proj_chain(dst_evac, lhsT_col, j, pool_tag):
                """Returns thunk fns for one 16-deep contraction chain.

                lhsT_col(t) -> AP for the stationary tile;
                dst_evac(ps) emits the evacuation."""
                ps_box = {}
                pool, tag = pool_tag

                def pair(q):
                    def emit():
                        if q == 0:
                            ps_box["ps"] = pool.tile(
                                [128, CH], f32, tag=tag, name="pps")
                        ps = ps_box["ps"]
                        for t in range(2 * q, 2 * q + 2):
                            nc.tensor.matmul(
                                ps[:], lhsT_col(t), xt_rhs(t, j),
                                start=(t == 0), stop=(t == NT - 1))
                        if q == 7:
                            dst_evac(ps)
                    return emit
                return [(pair(q), 600) for q in range(8)]

            def proj_thunks(j):
                # j=0 runs before any attention: rotate chains over the
                # (then free) attention accumulator banks so chains overlap
                # instead of serializing on one PSUM buffer.
                if j == 0:
                    rot = [(ps_proj, "proj"), (ps_pa, "paA"), (ps_pa, "paB")]
                else:
                    rot = [(ps_proj, "proj")]
                ch = [0]

                def nxt():
                    pt_ = rot[ch[0] % len(rot)]
                    ch[0] += 1
                    return pt_
                th = []

                def kev(ps):
                    nc.vector.tensor_copy(kt[j][:], ps[:])
                th += proj_chain(kev, lambda t: wk_t[:, t * 128:(t + 1) * 128],
                                 j, nxt())

                def vev(ps):
                    vt_s = wpool.tile([128, CH], bf16, tag="vt", name="vt_s")
                    nc.vector.tensor_copy(vt_s[:], ps[:])
                    for s in range(4):
                        nc.sync.dma_start_transpose(
                            v3[4 * j + s][:], vt_s[:, s * 128:(s + 1) * 128])
                th += proj_chain(vev, lambda t: wv_t[:, t * 128:(t + 1) * 128],
                                 j, nxt())
                for a in range(4):
                    def qev(ps, a=a):
                        nc.vector.tensor_copy(qt[a][j][:], ps[:])
                    th += proj_chain(
                        qev,
                        lambda t, a=a: wq_t[:, t * 512 + a * 128:
                                            t * 512 + (a + 1) * 128],
                        j, nxt())
                return th

            # ---- o_proj thunks ------------------------------------------
            def oproj_thunks(ci, an_tiles, tail=False):
                n0 = ci * CH
                th = []
                for nt_ in range(4):
                    for dc in range(4):
                        def emit(nt_=nt_, dc=dc):
                            # in the tail (post-attention) phase the score
                            # banks are free: rotate over them too so the
                            # PE never waits on a single bank's evacuation
                            if tail and (nt_ * 4 + dc) % 2:
                                po = ps_s.tile([128, 1024], f32,
                                               tag=f"s{(nt_ * 4 + dc) // 2 % 2}",
                                               name="po2")[:, 0:CH]
                            else:
                                po = ps_proj.tile([128, CH], f32, tag="proj",
                                                  name="po")
                            for a in range(4):
                                nc.tensor.matmul(
                                    po[:],
                                    an_tiles[a][:, nt_ * 128:(nt_ + 1) * 128],
                                    wo_t[:, a * D + dc * CH:
                                         a * D + (dc + 1) * CH],
                                    start=(a == 0), stop=(a == 3))
                            st = wpool.tile([128, CH], bf16, tag="st",
                                            bufs=3, name="st")
                            if (nt_ + dc) % 2 == 0:
                                nc.scalar.copy(st[:], po[:])
                            else:
                                nc.vector.tensor_copy(st[:], po[:])
                            nc.gpsimd.dma_start(
                                part.ap()[n0 + nt_ * 128:n0 + (nt_ + 1) * 128,
                                          dc * CH:(dc + 1) * CH],
                                st[:])
                        th.append((emit, 1400))
                return th

            # ---- attention ----------------------------------------------
            def attn_chunk(ci):
                n0 = ci * CH
                M = 4 * ci + 4
                an_tiles = []
                for g in range(2):
                    a0, a1 = 2 * g, 2 * g + 1
                    paA = ps_pa.tile([128, CH], f32, tag="paA", name="paA")
                    paB = ps_pa.tile([128, CH], f32, tag="paB", name="paB")
                    dn = ps_d.tile([128, CH], f32, tag="dn", name="dn")
                    pts = {}

                    def scores_exp(mt):
                        # score layout per head pair a: kv0 block at cols
                        # [0:F], kv1 block at cols [512:512+F] (bank 2);
                        # one FD=512+F exp covers both (cols [F:512] are
                        # junk for diagonal tiles and never read).
                        jmt, cmt = mt // 4, mt % 4
                        flo = max(0, (mt - 4 * ci) * 128)
                        F = CH - flo
                        # kv1 block always in bank 1: the two row-tiled
                        # score matmuls run concurrently and must not
                        # target the same PSUM bank (cols [F:CH] junk)
                        off = CH
                        sss, pt_pair = [], []
                        for i, a in enumerate((a0, a1)):
                            ss = ps_s.tile([128, 1024], f32, tag=f"s{i}",
                                           name="ss")
                            nc.tensor.matmul(
                                ss[:, 0:F],
                                kt[jmt][0:64, cmt * 128:(cmt + 1) * 128],
                                qt[a][ci][0:64, flo:CH],
                                start=True, stop=True, tile_position=(0, 0))
                            nc.tensor.matmul(
                                ss[:, off:off + F],
                                kt[jmt][64:128, cmt * 128:(cmt + 1) * 128],
                                qt[a][ci][64:128, flo:CH],
                                start=True, stop=True, tile_position=(64, 0))
                            sss.append(ss)
                        for i in range(2):
                            pt_ = wpool.tile([128, 1024], bf16, tag=f"pt{i}",
                                             name="pt")
                            nc.scalar.activation(pt_[:, 0:off + F],
                                                 sss[i][:, 0:off + F],
                                                 Exp, scale=0.125)
                            if mt >= 4 * ci:  # diagonal block: causal mask
                                nc.vector.tensor_mul(
                                    pt_[:, 0:128], pt_[:, 0:128], msk_t[:])
                                nc.vector.tensor_mul(
                                    pt_[:, off:off + 128],
                                    pt_[:, off:off + 128], msk_t[:])
                            pt_pair.append(pt_)
                        pts[mt] = (pt_pair, flo, F, off)

                    def av_denom(mt):
                        pt_pair, flo, F, off = pts.pop(mt)
                        first, last = (mt == 0), (mt == M - 1)
                        for i in range(2):
                            pa = paA if i == 0 else paB
                            pt_ = pt_pair[i]
                            nc.tensor.matmul(
                                pa[0:64, flo:CH], v3[mt][:, 0:64],
                                pt_[:, 0:F],
                                start=first, stop=last, tile_position=(0, 0))
                            nc.tensor.matmul(
                                pa[64:128, flo:CH], v3[mt][:, 64:128],
                                pt_[:, off:off + F],
                                start=first, stop=last, tile_position=(0, 64))
                        for pos, src in ((0, pt_pair[0][:, 0:F]),
                                         (32, pt_pair[0][:, off:off + F]),
                                         (64, pt_pair[1][:, 0:F]),
                                         (96, pt_pair[1][:, off:off + F])):
                            nc.tensor.matmul(
                                dn[pos:pos + 1, flo:CH], ones1[:], src,
                                start=first, stop=last,
                                tile_position=(0, pos))

                    for mt in range(M):
                        scores_exp(mt)
                        if mt > 0:
                            av_denom(mt - 1)
                        flo = max(0, (mt - 4 * ci) * 128)
                        F = CH - flo
                        slack = 2 * (2 * F + 352) / 1.2 - (5 * F / 2.4 + 400)
                        drain(max(0, slack))
                    av_denom(M - 1)

                    # Evacuate pa raw (frees the accumulator banks for the
                    # next group immediately); normalize off-critical-path.
                    aots = []
                    for i in range(2):
                        aot = wpool.tile([128, CH], bf16, tag=f"aot{i}",
                                         name="aot")
                        nc.vector.tensor_copy(aot[:], (paA if i == 0 else paB)[:])
                        aots.append(aot)
                    d4r = wpool.tile([128, CH], f32, tag="d4r", name="d4r")
                    nc.vector.reciprocal_approx_fast(d4r[0:97, :], dn[0:97, :])
                    # broadcast 1/denom rows to 128 partitions via K=1 PE
                    # matmuls into the (now free) score banks: all four in
                    # one window (pairwise-disjoint PE quadrants)
                    rbs = [ps_s.tile([128, 1024], f32, tag=f"s{i}",
                                     name="rb") for i in range(2)]
                    for i in range(2):
                        r0, r1 = 64 * i, 64 * i + 32
                        nc.tensor.matmul(rbs[i][0:64, 0:CH],
                                         onesb[r0:r0 + 1, :],
                                         d4r[r0:r0 + 1, :], start=True,
                                         stop=True, tile_position=(r0, 0))
                        nc.tensor.matmul(rbs[i][64:128, CH:2 * CH],
                                         onesb[r1:r1 + 1, :],
                                         d4r[r1:r1 + 1, :], start=True,
                                         stop=True, tile_position=(r1, 64))
                    for i, a in enumerate((a0, a1)):
                        rbr = wpool.tile([128, CH], f32, tag=f"rbr{i}",
                                         name="rbr")
                        nc.scalar.copy(rbr[0:64, :], rbs[i][0:64, 0:CH])
                        nc.scalar.copy(rbr[64:128, :],
                                       rbs[i][64:128, CH:2 * CH])
                        an = wpool.tile([128, CH], bf16, tag=f"an{a}",
                                        name=f"an{a}")
                        nc.vector.tensor_mul(an[:], aots[i][:], rbr[:])
                        an_tiles.append(an)
                return an_tiles

            # ---- main schedule ------------------------------------------
            for fn, _ in proj_thunks(0):
                fn()
            an_by_ci = {}
            for ci in range(NCHUNK):
                if ci < NCHUNK - 1:
                    fillers.extend(proj_thunks(ci + 1))
                if ci >= 1:
                    fillers.extend(oproj_thunks(ci - 1, an_by_ci[ci - 1]))
                an_by_ci[ci] = attn_chunk(ci)
                drain_all()
            for fn, _ in oproj_thunks(NCHUNK - 1, an_by_ci[NCHUNK - 1],
                                      tail=True):
                fn()
    nc.compile()
    return nc


def _prep_in_maps(x, Wq, Wk, Wv, Wo):
    import jax.numpy as jnp

    def to_bf16(a):
        return np.asarray(jnp.asarray(np.asarray(a), dtype=jnp.bfloat16))

    i = np.arange(128)[:, None]
    j = np.arange(128)[None, :]
    msk = (i <= j).astype(np.float32)

    def devlay(a):
        # [K*128, O] -> [128, K*O] partition-major device layout
        k = a.shape[0] // 128
        return a.reshape(k, 128, a.shape[1]).transpose(1, 0, 2).reshape(128, -1)

    in_maps = []
    for c in range(N_CORES):
        b, g = c // 4, c % 4
        qh = [8 * g + a for a in range(8)]
        wq_cols = []
        for a in range(4):
            wq_cols.append(np.arange(qh[a] * HD, (qh[a] + 1) * HD))
            wq_cols.append(np.arange(qh[a + 4] * HD, (qh[a + 4] + 1) * HD))
        wq_r = np.asarray(Wq)[:, np.concatenate(wq_cols)]
        wo_r = np.asarray(Wo)[np.concatenate(wq_cols), :]
        wk_s = np.asarray(Wk)[:, 2 * g * HD: (2 * g + 2) * HD]
        wv_s = np.asarray(Wv)[:, 2 * g * HD: (2 * g + 2) * HD]
        in_maps.append({
            "xT": to_bf16(devlay(np.ascontiguousarray(np.asarray(x)[b].T))),
            "wq": to_bf16(devlay(wq_r)),
            "wk": to_bf16(devlay(wk_s)),
            "wv": to_bf16(devlay(wv_s)),
            "wo": to_bf16(devlay(wo_r)),
            "msk": to_bf16(msk),
        })
    return in_maps


def kernel(x, Wq, Wk, Wv, Wo, trace=False):
    if "nc" not in _CACHE:
        _CACHE["nc"] = _build()
    nc = _CACHE["nc"]
    in_maps = _prep_in_maps(x, Wq, Wk, Wv, Wo)
    res = bass_utils.run_bass_kernel_spmd(
        nc, in_maps, core_ids=list(range(N_CORES)), trace=trace)
    _CACHE["last_result"] = res
    out = np.zeros((B, N, D), np.float32)
    for c in range(N_CORES):
        out[c // 4] += np.asarray(res.results[c]["part"], dtype=np.float32)
    return out


# revision 34
# speedup vs baseline: 1.2261x; 1.0103x over previous
"""GroupedQueryAttention forward on 8 Trainium2 NeuronCores (Bass/Tile), v2.

Sharding (per spec hint): data-parallel over batch (B=2) x tensor-parallel
over KV-head groups (4 groups of 2 KV heads + their 8 query heads each).
Core c -> (batch b = c // 4, group g = c % 4).

v2 redesign vs baseline (573us):
  - scores row-tiled 2x: the two K=64 (per-kv-head) score matmuls run
    concurrently in disjoint PE row groups (tile_position (0,0)/(64,0)).
  - attn@V col-tiled 2x: the two M=64 AV matmuls run concurrently in
    disjoint PE col groups ((0,0)/(0,64)), both kv heads in one PSUM bank.
  - softmax denominators via 4 concurrent M=1 matmuls (ones stationary)
    col-tiled into one PSUM bank; reciprocal via DVE approx; broadcast to
    128 partitions via stride-0-source DMA; normalize fused into the PSUM
    evacuation tensor_tensor.
  - causal triangle trimming: diagonal-block matmuls/exps only cover the
    columns right of the diagonal (query >= key block start).
  - exp batched to FD=2*F per ACTIVATE from bf16 score PSUM banks.
  - mt-outer loop reuses kT/V stationaries across the 2 head pairs of a
    group; V transposed via DMA xbar instead of the PE.
  - software-pipelined emission: projection and o_proj matmul groups are
    interleaved into the attention loop as PE fillers so the tensor engine
    never idles while the scalar engine chews exps (keeps HAM at K=8/8).
  - o_proj partials written as bf16 (halves output DMA); host sums fp32.

All device compute bf16 with fp32 PSUM accumulation (bf16 PSUM for raw
scores only, pre-exp). Host pre-casts/pre-transposes x and pre-slices and
pair-reorders the weight shards.
"""

from collections import deque

import numpy as np

import concourse.bass as bass  # noqa: F401  (import keeps engine registry warm)
import concourse.mybir as mybir
import concourse.tile as tile
from concourse import bacc, bass_utils

# Problem shape (hardcoded per contract).
B, N, D = 2, 2048, 2048
NUM_HEADS = 32
NUM_KV_HEADS = 8
HD = 64
G = NUM_HEADS // NUM_KV_HEADS  # 4
N_CORES = 8
NT = D // 128                  # 16 contraction tiles
NCHUNK = 4                     # token chunks of 512
CH = 512

_CACHE = {}


def _build():
    nc = bacc.Bacc("TRN2", target_bir_lowering=False, debug=False,
                   num_devices=N_CORES)
    f32, bf16 = mybir.dt.float32, mybir.dt.bfloat16
    Exp = mybir.ActivationFunctionType.Exp

    # all inputs pre-arranged host-side into the SBUF tile layout
    # (partition-major) so every load is contiguous per partition
    xT = nc.dram_tensor("xT", [128, NT * N], bf16, kind="ExternalInput")
    wq = nc.dram_tensor("wq", [128, NT * 512], bf16, kind="ExternalInput")
    wk = nc.dram_tensor("wk", [128, NT * 128], bf16, kind="ExternalInput")
    wv = nc.dram_tensor("wv", [128, NT * 128], bf16, kind="ExternalInput")
    wo = nc.dram_tensor("wo", [128, 4 * D], bf16, kind="ExternalInput")
    msk = nc.dram_tensor("msk", [128, 128], bf16, kind="ExternalInput")
    part = nc.dram_tensor("part", [N, D], bf16, kind="ExternalOutput")

    with tile.TileContext(nc) as tc:
        with (
            tc.tile_pool(name="const", bufs=1) as cpool,
            tc.tile_pool(name="proj", bufs=1) as kpool,
            tc.tile_pool(name="work", bufs=2) as wpool,
            tc.tile_pool(name="ps_s", bufs=1, space="PSUM") as ps_s,
            tc.tile_pool(name="ps_pa", bufs=1, space="PSUM") as ps_pa,
            tc.tile_pool(name="ps_d", bufs=1, space="PSUM") as ps_d,
            tc.tile_pool(name="ps_proj", bufs=1, space="PSUM") as ps_proj,
        ):
            # ---- inputs / constants -------------------------------------
            wk_t = cpool.tile([128, NT * 128], bf16, tag="wk")
            nc.sync.dma_start(wk_t[:], wk.ap()[:])
            wv_t = cpool.tile([128, NT * 128], bf16, tag="wv")
            nc.sync.dma_start(wv_t[:], wv.ap()[:])
            # wq/wo/msk ride other queues so they load in parallel with
            # the sync-queue xt chunks.
            wq_t = cpool.tile([128, NT * 512], bf16, tag="wq")
            nc.scalar.dma_start(wq_t[:], wq.ap()[:])
            wo_t = cpool.tile([128, 4 * D], bf16, tag="wo")
            nc.gpsimd.dma_start(wo_t[:], wo.ap()[:])
            msk_t = cpool.tile([128, 128], bf16, tag="msk")
            nc.gpsimd.dma_start(msk_t[:], msk.ap()[:])
            # x^T in 8 t-range chunks, fully contiguous per partition
            xtc = []
            for q in range(8):
                xq = cpool.tile([128, 2 * N], bf16, tag=f"xtc{q}")
                nc.sync.dma_start(xq[:],
                                  xT.ap()[:, 2 * q * N:(2 * q + 2) * N])
                xtc.append(xq)

            def xt_rhs(t, j):
                return xtc[t // 2][:, (t % 2) * N + j * CH:
                                   (t % 2) * N + (j + 1) * CH]
            ones1 = cpool.tile([128, 1], bf16, tag="ones1")
            nc.vector.memset(ones1[:], 1.0)
            onesb = cpool.tile([128, 64], f32, tag="onesb")
            nc.vector.memset(onesb[:], 1.0)
            # pre-warm the exp activation table while the DMAs run
            scr = cpool.tile([1, 8], f32, tag="scr")
            nc.vector.memset(scr[:], 0.0)
            nc.scalar.activation(scr[0:1, :], scr[0:1, :], Exp)

            # persistent projection outputs
            kt = [kpool.tile([128, CH], bf16, tag=f"kt{j}", name=f"kt{j}")
                  for j in range(NCHUNK)]
            v3 = [kpool.tile([128, 128], bf16, tag=f"v3_{m}", name=f"v3_{m}")
                  for m in range(N // 128)]
            qt = [[kpool.tile([128, CH], bf16, tag=f"qt{a}_{j}",
                              name=f"qt{a}_{j}")
                   for j in range(NCHUNK)] for a in range(4)]

            # ---- filler machinery ---------------------------------------
            fillers = deque()  # (emit_fn, est_pe_ns)

            def drain(budget_ns):
                spent = 0
                while fillers and spent < budget_ns:
                    fn, est = fillers.popleft()
                    fn()
                    spent += est

            def drain_all():
                while fillers:
                    fn, _ = fillers.popleft()
                    fn()

            # ---- projection thunks --------------------------------------
            def proj_chain(dst_evac, lhsT_col, j, alloc_ps):
                """Returns thunk fns for one 16-deep contraction chain.

                lhsT_col(t) -> AP for the stationary tile;
                dst_evac(ps) emits the evacuation."""
                ps_box = {}

                def pair(q):
                    def emit():
                        if q == 0:
                            ps_box["ps"] = alloc_ps()
                        ps = ps_box["ps"]
                        for t in range(2 * q, 2 * q + 2):
                            nc.tensor.matmul(
                                ps[:], lhsT_col(t), xt_rhs(t, j),
                                start=(t == 0), stop=(t == NT - 1))
                        if q == 7:
                            dst_evac(ps)
                    return emit
                return [(pair(q), 600) for q in range(8)]

            def proj_thunks(j):
                # j=0 runs before any attention: rotate the 6 chains over
                # the (then free) attention banks and emit chunk-major, so
                # every chain can progress as its xT chunk arrives.
                if j == 0:
                    rots = [
                        lambda: ps_proj.tile([128, CH], f32, tag="proj",
                                             name="pps"),
                        lambda: ps_pa.tile([128, CH], f32, tag="paA",
                                           name="pps"),
                        lambda: ps_pa.tile([128, CH], f32, tag="paB",
                                           name="pps"),
                        lambda: ps_d.tile([128, CH], f32, tag="dn",
                                          name="pps"),
                        lambda: ps_s.tile([128, 1024], f32, tag="s0",
                                          name="pps")[:, 0:CH],
                        lambda: ps_s.tile([128, 1024], f32, tag="s1",
                                          name="pps")[:, 0:CH],
                    ]
                else:
                    rots = [lambda: ps_proj.tile([128, CH], f32, tag="proj",
                                                 name="pps")]
                ch = [0]

                def nxt():
                    a = rots[ch[0] % len(rots)]
                    ch[0] += 1
                    return a
                chains = []

                def kev(ps):
                    nc.vector.tensor_copy(kt[j][:], ps[:])
                chains.append(proj_chain(
                    kev, lambda t: wk_t[:, t * 128:(t + 1) * 128], j, nxt()))

                def vev(ps):
                    vt_s = wpool.tile([128, CH], bf16, tag="vt", name="vt_s")
                    nc.vector.tensor_copy(vt_s[:], ps[:])
                    for s in range(4):
                        nc.sync.dma_start_transpose(
                            v3[4 * j + s][:], vt_s[:, s * 128:(s + 1) * 128])
                chains.append(proj_chain(
                    vev, lambda t: wv_t[:, t * 128:(t + 1) * 128], j, nxt()))
                for a in range(4):
                    def qev(ps, a=a):
                        nc.vector.tensor_copy(qt[a][j][:], ps[:])
                    chains.append(proj_chain(
                        qev,
                        lambda t, a=a: wq_t[:, t * 512 + a * 128:
                                            t * 512 + (a + 1) * 128],
                        j, nxt()))
                if j == 0:  # chunk-major: all chains advance per chunk
                    return [chains[c][q] for q in range(8) for c in range(6)]
                return [th for chain in chains for th in chain]

            # ---- o_proj thunks ------------------------------------------
            def oproj_thunks(ci, an_tiles, tail=False):
                n0 = ci * CH
                th = []
                for nt_ in range(4):
                    for dc in range(4):
                        def emit(nt_=nt_, dc=dc):
                            # in the tail (post-attention) phase the score
                            # banks are free: rotate over them too so the
                            # PE never waits on a single bank's evacuation
                            if tail and (nt_ * 4 + dc) % 2:
                                po = ps_s.tile([128, 1024], f32,
                                               tag=f"s{(nt_ * 4 + dc) // 2 % 2}",
                                               name="po2")[:, 0:CH]
                            else:
                                po = ps_proj.tile([128, CH], f32, tag="proj",
                                                  name="po")
                            for a in range(4):
                                nc.tensor.matmul(
                                    po[:],
                                    an_tiles[a][:, nt_ * 128:(nt_ + 1) * 128],
                                    wo_t[:, a * D + dc * CH:
                                         a * D + (dc + 1) * CH],
                                    start=(a == 0), stop=(a == 3))
                            st = wpool.tile([128, CH], bf16, tag="st",
                                            bufs=3, name="st")
                            if (nt_ + dc) % 2 == 0:
                                nc.scalar.copy(st[:], po[:])
                            else:
                                nc.vector.tensor_copy(st[:], po[:])
                            nc.gpsimd.dma_start(
                                part.ap()[n0 + nt_ * 128:n0 + (nt_ + 1) * 128,
                                          dc * CH:(dc + 1) * CH],
                                st[:])
                        th.append((emit, 1400))
                return th

            # ---- attention ----------------------------------------------
            def attn_chunk(ci):
                n0 = ci * CH
                M = 4 * ci + 4
                an_tiles = []
                for g in range(2):
                    a0, a1 = 2 * g, 2 * g + 1
                    paA = ps_pa.tile([128, CH], f32, tag="paA", name="paA")
                    paB = ps_pa.tile([128, CH], f32, tag="paB", name="paB")
                    dn = ps_d.tile([128, CH], f32, tag="dn", name="dn")
                    pts = {}

                    def scores_exp(mt):
                        # score layout per head pair a: kv0 block at cols
                        # [0:F], kv1 block at cols [512:512+F] (bank 2);
                        # one FD=512+F exp covers both (cols [F:512] are
                        # junk for diagonal tiles and never read).
                        jmt, cmt = mt // 4, mt % 4
                        flo = max(0, (mt - 4 * ci) * 128)
                        F = CH - flo
                        # kv1 block always in bank 1: the two row-tiled
                        # score matmuls run concurrently and must not
                        # target the same PSUM bank (cols [F:CH] junk)
                        off = CH
                        sss, pt_pair = [], []
                        for i, a in enumerate((a0, a1)):
                            ss = ps_s.tile([128, 1024], f32, tag=f"s{i}",
                                           name="ss")
                            nc.tensor.matmul(
                                ss[:, 0:F],
                                kt[jmt][0:64, cmt * 128:(cmt + 1) * 128],
                                qt[a][ci][0:64, flo:CH],
                                start=True, stop=True, tile_position=(0, 0))
                            nc.tensor.matmul(
                                ss[:, off:off + F],
                                kt[jmt][64:128, cmt * 128:(cmt + 1) * 128],
                                qt[a][ci][64:128, flo:CH],
                                start=True, stop=True, tile_position=(64, 0))
                            sss.append(ss)
                        for i in range(2):
                            pt_ = wpool.tile([128, 1024], bf16, tag=f"pt{i}",
                                             name="pt")
                            nc.scalar.activation(pt_[:, 0:off + F],
                                                 sss[i][:, 0:off + F],
                                                 Exp, scale=0.125)
                            if mt >= 4 * ci:  # diagonal block: causal mask
                                nc.vector.tensor_mul(
                                    pt_[:, 0:128], pt_[:, 0:128], msk_t[:])
                                nc.vector.tensor_mul(
                                    pt_[:, off:off + 128],
                                    pt_[:, off:off + 128], msk_t[:])
                            pt_pair.append(pt_)
                        pts[mt] = (pt_pair, flo, F, off)

                    def av_denom(mt):
                        pt_pair, flo, F, off = pts.pop(mt)
                        first, last = (mt == 0), (mt == M - 1)
                        for i in range(2):
                            pa = paA if i == 0 else paB
                            pt_ = pt_pair[i]
                            nc.tensor.matmul(
                                pa[0:64, flo:CH], v3[mt][:, 0:64],
                                pt_[:, 0:F],
                                start=first, stop=last, tile_position=(0, 0))
                            nc.tensor.matmul(
                                pa[64:128, flo:CH], v3[mt][:, 64:128],
                                pt_[:, off:off + F],
                                start=first, stop=last, tile_position=(0, 64))
                        for pos, src in ((0, pt_pair[0][:, 0:F]),
                                         (32, pt_pair[0][:, off:off + F]),
                                         (64, pt_pair[1][:, 0:F]),
                                         (96, pt_pair[1][:, off:off + F])):
                            nc.tensor.matmul(
                                dn[pos:pos + 1, flo:CH], ones1[:], src,
                                start=first, stop=last,
                                tile_position=(0, pos))

                    for mt in range(M):
                        scores_exp(mt)
                        if mt > 0:
                            av_denom(mt - 1)
                        flo = max(0, (mt - 4 * ci) * 128)
                        F = CH - flo
                        slack = 2 * (2 * F + 352) / 1.2 - (5 * F / 2.4 + 400)
                        drain(max(0, slack))
                    av_denom(M - 1)

                    # Evacuate pa raw (frees the accumulator banks for the
                    # next group immediately); normalize off-critical-path.
                    aots = []
                    for i in range(2):
                        aot = wpool.tile([128, CH], bf16, tag=f"aot{i}",
                                         name="aot")
                        nc.vector.tensor_copy(aot[:], (paA if i == 0 else paB)[:])
                        aots.append(aot)
                    d4r = wpool.tile([128, CH], f32, tag="d4r", name="d4r")
                    nc.vector.reciprocal_approx_fast(d4r[0:97, :], dn[0:97, :])
                    # broadcast 1/denom rows to 128 partitions via K=1 PE
                    # matmuls into the (now free) score banks: all four in
                    # one window (pairwise-disjoint PE quadrants)
                    rbs = [ps_s.tile([128, 1024], f32, tag=f"s{i}",
                                     name="rb") for i in range(2)]
                    for i in range(2):
                        r0, r1 = 64 * i, 64 * i + 32
                        nc.tensor.matmul(rbs[i][0:64, 0:CH],
                                         onesb[r0:r0 + 1, :],
                                         d4r[r0:r0 + 1, :], start=True,
                                         stop=True, tile_position=(r0, 0))
                        nc.tensor.matmul(rbs[i][64:128, CH:2 * CH],
                                         onesb[r1:r1 + 1, :],
                                         d4r[r1:r1 + 1, :], start=True,
                                         stop=True, tile_position=(r1, 64))
                    for i, a in enumerate((a0, a1)):
                        rbr = wpool.tile([128, CH], f32, tag=f"rbr{i}",
                                         name="rbr")
                        nc.scalar.copy(rbr[0:64, :], rbs[i][0:64, 0:CH])
                        nc.scalar.copy(rbr[64:128, :],
                                       rbs[i][64:128, CH:2 * CH])
                        an = wpool.tile([128, CH], bf16, tag=f"an{a}",
                                        name=f"an{a}")
                        nc.vector.tensor_mul(an[:], aots[i][:], rbr[:])
                        an_tiles.append(an)
                return an_tiles

            # ---- main schedule ------------------------------------------
            for fn, _ in proj_thunks(0):
                fn()
            an_by_ci = {}
            for ci in range(NCHUNK):
                if ci < NCHUNK - 1:
                    fillers.extend(proj_thunks(ci + 1))
                if ci >= 1:
                    fillers.extend(oproj_thunks(ci - 1, an_by_ci[ci - 1]))
                an_by_ci[ci] = attn_chunk(ci)
                drain_all()
            for fn, _ in oproj_thunks(NCHUNK - 1, an_by_ci[NCHUNK - 1],
                                      tail=True):
                fn()
    nc.compile()
    return nc


def _prep_in_maps(x, Wq, Wk, Wv, Wo):
    import jax.numpy as jnp

    def to_bf16(a):
        return np.asarray(jnp.asarray(np.asarray(a), dtype=jnp.bfloat16))

    i = np.arange(128)[:, None]
    j = np.arange(128)[None, :]
    msk = (i <= j).astype(np.float32)

    def devlay(a):
        # [K*128, O] -> [128, K*O] partition-major device layout
        k = a.shape[0] // 128
        return a.reshape(k, 128, a.shape[1]).transpose(1, 0, 2).reshape(128, -1)

    in_maps = []
    for c in range(N_CORES):
        b, g = c // 4, c % 4
        qh = [8 * g + a for a in range(8)]
        wq_cols = []
        for a in range(4):
            wq_cols.append(np.arange(qh[a] * HD, (qh[a] + 1) * HD))
            wq_cols.append(np.arange(qh[a + 4] * HD, (qh[a + 4] + 1) * HD))
        wq_r = np.asarray(Wq)[:, np.concatenate(wq_cols)]
        wo_r = np.asarray(Wo)[np.concatenate(wq_cols), :]
        wk_s = np.asarray(Wk)[:, 2 * g * HD: (2 * g + 2) * HD]
        wv_s = np.asarray(Wv)[:, 2 * g * HD: (2 * g + 2) * HD]
        in_maps.append({
            "xT": to_bf16(devlay(np.ascontiguousarray(np.asarray(x)[b].T))),
            "wq": to_bf16(devlay(wq_r)),
            "wk": to_bf16(devlay(wk_s)),
            "wv": to_bf16(devlay(wv_s)),
            "wo": to_bf16(devlay(wo_r)),
            "msk": to_bf16(msk),
        })
    return in_maps


def kernel(x, Wq, Wk, Wv, Wo, trace=False):
    if "nc" not in _CACHE:
        _CACHE["nc"] = _build()
    nc = _CACHE["nc"]
    in_maps = _prep_in_maps(x, Wq, Wk, Wv, Wo)
    res = bass_utils.run_bass_kernel_spmd(
        nc, in_maps, core_ids=list(range(N_CORES)), trace=trace)
    _CACHE["last_result"] = res
    out = np.zeros((B, N, D), np.float32)
    for c in range(N_CORES):
        out[c // 4] += np.asarray(res.results[c]["part"], dtype=np.float32)
    return out
